# revision 1
# baseline (speedup 1.0000x reference)
"""GNN message-passing layer (GCN w/ edge-feature attention) on 8 trn2 cores.

Math (per graph b, N=512 nodes, E=8 edge feats, D=64):
    pre_sup = x_b @ W                                   [N, D]
    s[i,j]  = sum_e coef[e] * edge[b,i,j,e]             [N, N]
    adj     = softmax(s / tau, axis=-1)   (tau = 1.0)
    adj_hat = adj + I;  d = rowsum(adj_hat) = 2 exactly (softmax rows sum to 1)
    out     = relu(0.5 * adj_hat @ pre_sup)
            = relu( (P @ (0.5*pre_sup)) / Z + 0.5*pre_sup )
  where P = exp(s) (unnormalized, no max-subtraction needed: |s| <~ 25),
  Z_i = sum_j P[i,j] obtained for free as an extra ones-column in the
  aggregation matmul.

Device mapping (per core: 8 graphs, 64 MiB of edge data = the roofline):
  - scores (default v3b): 5 of the 8 e-terms as PSUM-accumulated PE matmuls
    (lhsT = coef[e]*I_128, rhs = stride-8 e-slice; fp32 matmul is 4 cyc/row,
    so splitting engines beats PE-only); the other 3 e-terms as ACT
    scaled-copies (per-partition scale AP = coef[e]) + DVE tree adds.
    NB: DVE tensor_tensor must not mix PSUM+SBUF operands (HW fault) -- the
    PE partial is tensor_copy'd out of PSUM before the final add.
  - exp: ACT engine (no max-subtraction needed; |scores| < ~25).
  - transpose P tiles on PE (is_transpose matmul vs identity), copy to SBUF
    on DVE, then aggregation matmuls contract j with rhs=[0.5*pre_sup | 1];
    the ones column yields the softmax denominator Z for free.
  - finals: reciprocal + per-partition scale + skip add + relu, DMA out.
  Cost-model (TimelineSim): ~238 us/iter, DMA-bound (DMA 200, PE 172,
  DVE 130, ACT 95 us busy); v1 (PE-only scores) was 269 us, PE-bound.
"""

import os
from contextlib import ExitStack

import numpy as np

import concourse.bass as bass
import concourse.tile as tile
from concourse import bacc, mybir
from concourse.bass_utils import run_bass_kernel_spmd

F32 = mybir.dt.float32

B, N, E, D = 64, 512, 8, 64
NCORES = 8
BPC = B // NCORES          # graphs per core
PT = 128                   # partition tile (i-rows per edge tile)
NIT = N // PT              # 4 i-tiles (and j-chunks) per graph
TAU = 1.0

# Module-level knobs (test.py pokes these)
TRACE = os.environ.get("KERNEL_TRACE", "") == "1"
VARIANT = os.environ.get("KERNEL_VARIANT", "v3b")
LAST_RESULT = None

_nc_cache = {}


def _build_kernel(ctx: ExitStack, tc: "tile.TileContext", edge, x, w, cdiag, ident, out,
                  scores_f32r: bool = False, pe_e: int = E, coefb=None,
                  edge_bufs: int = 4):
    """pe_e: how many of the E per-edge-feature score terms run as PE matmuls;
    the remaining E-pe_e run as ACT scaled-copies + DVE tree adds (fp32)."""
    nc = tc.nc
    EDT = mybir.dt.float32r if scores_f32r else F32

    consts = ctx.enter_context(tc.tile_pool(name="consts", bufs=1))
    if pe_e < E:
        acc_pool = ctx.enter_context(tc.tile_pool(name="acc", bufs=2))
        cb = consts.tile([PT, E], F32)
        nc.sync.dma_start(cb[:], coefb[:])
    edge_pool = ctx.enter_context(tc.tile_pool(name="edge", bufs=edge_bufs))
    xt_pool = ctx.enter_context(tc.tile_pool(name="xt", bufs=2))
    xT_pool = ctx.enter_context(tc.tile_pool(name="xT", bufs=2))
    psup_pool = ctx.enter_context(tc.tile_pool(name="psup", bufs=2))
    p_pool = ctx.enter_context(tc.tile_pool(name="p", bufs=2))
    pT_pool = ctx.enter_context(tc.tile_pool(name="pT", bufs=2))
    fin_pool = ctx.enter_context(tc.tile_pool(name="fin", bufs=3))
    o_pool = ctx.enter_context(tc.tile_pool(name="o", bufs=3))

    misc_psum = ctx.enter_context(tc.tile_pool(name="mpsum", bufs=2, space="PSUM"))
    sc_psum = ctx.enter_context(tc.tile_pool(name="scpsum", bufs=2, space="PSUM"))
    pT_psum = ctx.enter_context(tc.tile_pool(name="ptpsum", bufs=2, space="PSUM"))
    out_psum = ctx.enter_context(tc.tile_pool(name="opsum", bufs=2, space="PSUM"))

    # Constants
    cd = consts.tile([PT, E * PT], EDT)       # cd[:, e*128:(e+1)*128] = coef[e] * I
    if scores_f32r:
        nc.gpsimd.dma_start(cd[:], cdiag[:])  # SWDGE casts f32 -> f32r inline
    else:
        nc.sync.dma_start(cd[:], cdiag[:])
    idn = consts.tile([PT, PT], F32)
    nc.sync.dma_start(idn[:], ident[:])
    wsb = consts.tile([D, D], F32)
    nc.sync.dma_start(wsb[:], w[:])

    x_r = x[:].rearrange("(b it p) d -> b p it d", b=BPC, it=NIT, p=PT)

    def compute_psup(b):
        """pre_sup' = 0.5 * (x_b @ W) with a trailing ones column per j-chunk."""
        xt = xt_pool.tile([PT, NIT * D], F32)
        nc.sync.dma_start(xt[:].rearrange("p (it d) -> p it d", it=NIT), x_r[b])
        psup = psup_pool.tile([PT, NIT * (D + 1)], F32)
        for it in range(NIT):
            xT_ps = misc_psum.tile([D, PT], F32, tag="m")
            nc.tensor.matmul(xT_ps[:], xt[:, it * D:(it + 1) * D], idn[:],
                             is_transpose=True)
            xT_sb = xT_pool.tile([D, PT], F32)
            nc.vector.tensor_copy(xT_sb[:], xT_ps[:])
            ps_ps = misc_psum.tile([PT, D], F32, tag="m")
            nc.tensor.matmul(ps_ps[:], xT_sb[:], wsb[:], start=True, stop=True)
            nc.scalar.mul(psup[:, it * (D + 1):it * (D + 1) + D], ps_ps[:], 0.5)
            nc.vector.memset(psup[:, it * (D + 1) + D:(it + 1) * (D + 1)], 1.0)
        return psup

    def scores_tile(b, it):
        """DMA one edge tile and run the 8 accumulating score matmuls."""
        et = edge_pool.tile([PT, N * E], EDT)
        row0 = b * N + it * PT
        if scores_f32r:
            nc.gpsimd.dma_start(et[:], edge[row0:row0 + PT, :])
        else:
            nc.sync.dma_start(et[:], edge[row0:row0 + PT, :])
        et3 = et[:].rearrange("p (j e) -> p j e", e=E)
        sc_ps = sc_psum.tile([PT, N], F32)
        for e in range(pe_e):
            nc.tensor.matmul(sc_ps[:], cd[:, e * PT:(e + 1) * PT], et3[:, :, e],
                             start=(e == 0), stop=(e == pe_e - 1))
        p_sb = p_pool.tile([PT, N], F32)
        if pe_e == E:
            nc.scalar.activation(p_sb[:], sc_ps[:],
                                 mybir.ActivationFunctionType.Exp, scale=1.0 / TAU)
        else:
            # ACT: t_e = coef[e] * edge[:, :, e]; DVE: tree-add + fold in PSUM.
            ts = []
            for e in range(pe_e, E):
                t = acc_pool.tile([PT, N], F32, tag=f"t{e - pe_e}")
                nc.scalar.activation(t[:], et3[:, :, e],
                                     mybir.ActivationFunctionType.Copy,
                                     scale=cb[:, e:e + 1])
                ts.append(t)
            s = acc_pool.tile([PT, N], F32, tag="s0")
            nc.vector.tensor_add(s[:], ts[0][:], ts[1][:])
            for k, t in enumerate(ts[2:]):
                s2 = acc_pool.tile([PT, N], F32, tag=f"s{k + 1}")
                nc.vector.tensor_add(s2[:], s[:], t[:])
                s = s2
            # DVE tensor_tensor must not mix PSUM+SBUF operands (HW fault):
            # copy the PE partial out of PSUM first, then add SBUF+SBUF.
            sc_sb = acc_pool.tile([PT, N], F32, tag="scsb")
            nc.vector.tensor_copy(sc_sb[:], sc_ps[:])
            sf = acc_pool.tile([PT, N], F32, tag="sf")
            nc.vector.tensor_add(sf[:], sc_sb[:], s[:])
            nc.scalar.activation(p_sb[:], sf[:],
                                 mybir.ActivationFunctionType.Exp, scale=1.0 / TAU)
        return p_sb

    def post_tile(b, it, p_sb, psup):
        """Transpose P, aggregate against pre_sup'+ones, normalize, relu, store."""
        pT_sb = pT_pool.tile([PT, N], F32)
        for jc in range(NIT):
            pT_ps = pT_psum.tile([PT, PT], F32)
            nc.tensor.matmul(pT_ps[:], p_sb[:, jc * PT:(jc + 1) * PT], idn[:],
                             is_transpose=True)
            nc.vector.tensor_copy(pT_sb[:, jc * PT:(jc + 1) * PT], pT_ps[:])
        o_ps = out_psum.tile([PT, D + 1], F32)
        for jc in range(NIT):
            nc.tensor.matmul(o_ps[:], pT_sb[:, jc * PT:(jc + 1) * PT],
                             psup[:, jc * (D + 1):(jc + 1) * (D + 1)],
                             start=(jc == 0), stop=(jc == NIT - 1))
        r = fin_pool.tile([PT, 1], F32, tag="r")
        nc.vector.reciprocal(r[:], o_ps[:, D:D + 1])
        t1 = fin_pool.tile([PT, D], F32, tag="t1")
        nc.vector.tensor_scalar_mul(t1[:], o_ps[:, 0:D], r[:])
        t2 = fin_pool.tile([PT, D], F32, tag="t2")
        nc.vector.tensor_add(t2[:], t1[:],
                             psup[:, it * (D + 1):it * (D + 1) + D])
        o_sb = o_pool.tile([PT, D], F32)
        nc.scalar.activation(o_sb[:], t2[:], mybir.ActivationFunctionType.Relu)
        row0 = b * N + it * PT
        nc.sync.dma_start(out[row0:row0 + PT, :], o_sb[:])

    # Software-pipelined emission: post(k-1) lands between scores(k) and
    # scores(k+1) so the PE never waits on ACT's exp.
    pending = None
    for b in range(BPC):
        psup = compute_psup(b)
        for it in range(NIT):
            p_sb = scores_tile(b, it)
            if pending is not None:
                post_tile(*pending)
            pending = (b, it, p_sb, psup)
    post_tile(*pending)


def _build_dma_only(ctx: ExitStack, tc: "tile.TileContext", edge, x, w, cdiag, ident, out,
                    coefb=None):
    """Variant: just the edge DMA stream + a trivial out write (BW probe)."""
    nc = tc.nc
    edge_pool = ctx.enter_context(tc.tile_pool(name="edge", bufs=4))
    o_pool = ctx.enter_context(tc.tile_pool(name="o", bufs=2))
    for b in range(BPC):
        for it in range(NIT):
            et = edge_pool.tile([PT, N * E], F32)
            row0 = b * N + it * PT
            nc.sync.dma_start(et[:], edge[row0:row0 + PT, :])
            o_sb = o_pool.tile([PT, D], F32)
            nc.vector.tensor_copy(o_sb[:], et[:, 0:D])
            nc.sync.dma_start(out[row0:row0 + PT, :], o_sb[:])


def _build_kernel_f32r(ctx, tc, edge, x, w, cdiag, ident, out, coefb=None):
    _build_kernel(ctx, tc, edge, x, w, cdiag, ident, out, scores_f32r=True)


def _build_kernel_split4(ctx, tc, edge, x, w, cdiag, ident, out, coefb=None):
    _build_kernel(ctx, tc, edge, x, w, cdiag, ident, out, pe_e=4, coefb=coefb)


def _build_kernel_split5(ctx, tc, edge, x, w, cdiag, ident, out, coefb=None):
    _build_kernel(ctx, tc, edge, x, w, cdiag, ident, out, pe_e=5, coefb=coefb)


def _build_kernel_v4(ctx, tc, edge, x, w, cdiag, ident, out, coefb=None):
    _build_kernel(ctx, tc, edge, x, w, cdiag, ident, out, pe_e=5, coefb=coefb,
                  edge_bufs=6)


_BUILDERS = {"v1": _build_kernel, "v2": _build_kernel_f32r, "dma": _build_dma_only,
             "v3": _build_kernel_split4, "v3b": _build_kernel_split5,
             "v4": _build_kernel_v4}


def _get_nc(reps: int = 1, variant: str = "v1"):
    key = f"{variant}-r{reps}"
    internal_edge = variant.endswith("i")
    base_variant = variant[:-1] if internal_edge else variant
    if key not in _nc_cache:
        nc = bacc.Bacc("TRN2", target_bir_lowering=False, debug=False,
                       num_devices=NCORES)
        if internal_edge:
            # Bench-only: edge lives in device DRAM (uninitialized) so the
            # axon tunnel doesn't re-ship 512 MiB per timed call.
            edge = nc.dram_tensor("edge_int", [BPC * N, N * E], F32)
        else:
            edge = nc.declare_dram_parameter("edge", [BPC * N, N * E], F32, isOutput=False)
        x = nc.declare_dram_parameter("x", [BPC * N, D], F32, isOutput=False)
        w = nc.declare_dram_parameter("w", [D, D], F32, isOutput=False)
        cdiag = nc.declare_dram_parameter("cdiag", [PT, E * PT], F32, isOutput=False)
        ident = nc.declare_dram_parameter("ident", [PT, PT], F32, isOutput=False)
        coefb = nc.declare_dram_parameter("coefb", [PT, E], F32, isOutput=False)
        out = nc.declare_dram_parameter("out", [BPC * N, D], F32, isOutput=True)
        builder = _BUILDERS[base_variant]
        with tile.TileContext(nc) as tc:
            for _ in range(reps):
                with ExitStack() as ctx:
                    builder(ctx, tc, edge, x, w, cdiag, ident, out, coefb=coefb)
        nc.compile()
        _nc_cache[key] = nc
    return _nc_cache[key]


def kernel(**inputs) -> np.ndarray:
    global LAST_RESULT
    edge = np.ascontiguousarray(inputs["edge_features"], dtype=np.float32)
    x = np.ascontiguousarray(inputs["x"], dtype=np.float32)
    W = np.ascontiguousarray(inputs["W"], dtype=np.float32)
    coef = np.asarray(inputs["coef"], dtype=np.float32)

    c = coef[:, 0]
    cdiag = np.zeros((PT, E * PT), np.float32)
    ar = np.arange(PT)
    for e in range(E):
        cdiag[ar, e * PT + ar] = c[e]
    ident = np.eye(PT, dtype=np.float32)

    nc = _get_nc(variant=VARIANT)
    in_maps = []
    for core in range(NCORES):
        b0 = core * BPC
        in_maps.append({
            "edge": edge[b0:b0 + BPC].reshape(BPC * N, N * E),
            "x": x[b0 * N:(b0 + BPC) * N],
            "w": W,
            "cdiag": cdiag,
            "ident": ident,
            "coefb": np.repeat(c[None, :], PT, axis=0),
        })
    res = run_bass_kernel_spmd(nc, in_maps, list(range(NCORES)), trace=TRACE)
    LAST_RESULT = res
    return np.concatenate([res.results[i]["out"] for i in range(NCORES)], axis=0)



# revision 9
# speedup vs baseline: 1.4571x; 1.4571x over previous
"""GNN message-passing layer (GCN w/ edge-feature attention) on 8 trn2 cores.

Math (per graph b, N=512 nodes, E=8 edge feats, D=64):
    pre_sup = x_b @ W                                   [N, D]
    s[i,j]  = sum_e coef[e] * edge[b,i,j,e]             [N, N]
    adj     = softmax(s / tau, axis=-1)   (tau = 1.0)
    adj_hat = adj + I;  d = rowsum(adj_hat) = 2 exactly (softmax rows sum to 1)
    out     = relu(0.5 * adj_hat @ pre_sup)
            = relu( (P @ (0.5*pre_sup)) / Z + 0.5*pre_sup )
  where P = exp(s) (unnormalized, no max-subtraction needed: |s| <~ 25),
  Z_i = sum_j P[i,j] obtained for free as an extra ones-column in the
  aggregation matmul.

Device mapping (per core: 8 graphs, 64 MiB of edge data = the roofline):
  - scores (default v3b): 5 of the 8 e-terms as PSUM-accumulated PE matmuls
    (lhsT = coef[e]*I_128, rhs = stride-8 e-slice; fp32 matmul is 4 cyc/row,
    so splitting engines beats PE-only); the other 3 e-terms as ACT
    scaled-copies (per-partition scale AP = coef[e]) + DVE tree adds.
    NB: DVE tensor_tensor must not mix PSUM+SBUF operands (HW fault) -- the
    PE partial is tensor_copy'd out of PSUM before the final add.
  - exp: ACT engine (no max-subtraction needed; |scores| < ~25).
  - transpose P tiles on PE (is_transpose matmul vs identity), copy to SBUF
    on DVE, then aggregation matmuls contract j with rhs=[0.5*pre_sup | 1];
    the ones column yields the softmax denominator Z for free.
  - finals: reciprocal + per-partition scale + skip add + relu, DMA out.
  Cost-model (TimelineSim): ~238 us/iter, DMA-bound (DMA 200, PE 172,
  DVE 130, ACT 95 us busy); v1 (PE-only scores) was 269 us, PE-bound.
"""

import os
from contextlib import ExitStack

import ml_dtypes
import numpy as np

import concourse.bass as bass
import concourse.tile as tile
from concourse import bacc, mybir
from concourse.bass_utils import run_bass_kernel_spmd

F32 = mybir.dt.float32

B, N, E, D = 64, 512, 8, 64
NCORES = 8
BPC = B // NCORES          # graphs per core
PT = 128                   # partition tile (i-rows per edge tile)
NIT = N // PT              # 4 i-tiles (and j-chunks) per graph
TAU = 1.0

# Module-level knobs (test.py pokes these)
TRACE = os.environ.get("KERNEL_TRACE", "") == "1"
VARIANT = os.environ.get("KERNEL_VARIANT", "v3b")
LAST_RESULT = None

_nc_cache = {}


def _build_kernel(ctx: ExitStack, tc: "tile.TileContext", edge, x, w, cdiag, ident, out,
                  scores_f32r: bool = False, pe_e: int = E, coefb=None,
                  identb=None, edge_bufs: int = 4):
    """pe_e: how many of the E per-edge-feature score terms run as PE matmuls;
    the remaining E-pe_e run as ACT scaled-copies + DVE tree adds (fp32)."""
    nc = tc.nc
    EDT = mybir.dt.float32r if scores_f32r else F32

    consts = ctx.enter_context(tc.tile_pool(name="consts", bufs=1))
    if pe_e < E:
        acc_pool = ctx.enter_context(tc.tile_pool(name="acc", bufs=2))
        cb = consts.tile([PT, E], F32)
        nc.sync.dma_start(cb[:], coefb[:])
    edge_pool = ctx.enter_context(tc.tile_pool(name="edge", bufs=edge_bufs))
    xt_pool = ctx.enter_context(tc.tile_pool(name="xt", bufs=2))
    xT_pool = ctx.enter_context(tc.tile_pool(name="xT", bufs=2))
    psup_pool = ctx.enter_context(tc.tile_pool(name="psup", bufs=2))
    p_pool = ctx.enter_context(tc.tile_pool(name="p", bufs=2))
    pT_pool = ctx.enter_context(tc.tile_pool(name="pT", bufs=2))
    fin_pool = ctx.enter_context(tc.tile_pool(name="fin", bufs=3))
    o_pool = ctx.enter_context(tc.tile_pool(name="o", bufs=3))

    misc_psum = ctx.enter_context(tc.tile_pool(name="mpsum", bufs=2, space="PSUM"))
    sc_psum = ctx.enter_context(tc.tile_pool(name="scpsum", bufs=2, space="PSUM"))
    pT_psum = ctx.enter_context(tc.tile_pool(name="ptpsum", bufs=2, space="PSUM"))
    out_psum = ctx.enter_context(tc.tile_pool(name="opsum", bufs=2, space="PSUM"))

    # Constants
    cd = consts.tile([PT, E * PT], EDT)       # cd[:, e*128:(e+1)*128] = coef[e] * I
    if scores_f32r:
        nc.gpsimd.dma_start(cd[:], cdiag[:])  # SWDGE casts f32 -> f32r inline
    else:
        nc.sync.dma_start(cd[:], cdiag[:])
    idn = consts.tile([PT, PT], F32)
    nc.sync.dma_start(idn[:], ident[:])
    wsb = consts.tile([D, D], F32)
    nc.sync.dma_start(wsb[:], w[:])

    x_r = x[:].rearrange("(b it p) d -> b p it d", b=BPC, it=NIT, p=PT)

    def compute_psup(b):
        """pre_sup' = 0.5 * (x_b @ W) with a trailing ones column per j-chunk."""
        xt = xt_pool.tile([PT, NIT * D], F32)
        nc.sync.dma_start(xt[:].rearrange("p (it d) -> p it d", it=NIT), x_r[b])
        psup = psup_pool.tile([PT, NIT * (D + 1)], F32)
        for it in range(NIT):
            xT_ps = misc_psum.tile([D, PT], F32, tag="m")
            nc.tensor.matmul(xT_ps[:], xt[:, it * D:(it + 1) * D], idn[:],
                             is_transpose=True)
            xT_sb = xT_pool.tile([D, PT], F32)
            nc.vector.tensor_copy(xT_sb[:], xT_ps[:])
            ps_ps = misc_psum.tile([PT, D], F32, tag="m")
            nc.tensor.matmul(ps_ps[:], xT_sb[:], wsb[:], start=True, stop=True)
            nc.scalar.mul(psup[:, it * (D + 1):it * (D + 1) + D], ps_ps[:], 0.5)
            nc.vector.memset(psup[:, it * (D + 1) + D:(it + 1) * (D + 1)], 1.0)
        return psup

    def scores_tile(b, it):
        """DMA one edge tile and run the 8 accumulating score matmuls."""
        et = edge_pool.tile([PT, N * E], EDT)
        row0 = b * N + it * PT
        if scores_f32r:
            nc.gpsimd.dma_start(et[:], edge[row0:row0 + PT, :])
        else:
            nc.sync.dma_start(et[:], edge[row0:row0 + PT, :])
        et3 = et[:].rearrange("p (j e) -> p j e", e=E)
        sc_ps = sc_psum.tile([PT, N], F32)
        for e in range(pe_e):
            nc.tensor.matmul(sc_ps[:], cd[:, e * PT:(e + 1) * PT], et3[:, :, e],
                             start=(e == 0), stop=(e == pe_e - 1))
        p_sb = p_pool.tile([PT, N], F32)
        if pe_e == E:
            nc.scalar.activation(p_sb[:], sc_ps[:],
                                 mybir.ActivationFunctionType.Exp, scale=1.0 / TAU)
        else:
            # ACT: t_e = coef[e] * edge[:, :, e]; DVE: tree-add + fold in PSUM.
            ts = []
            for e in range(pe_e, E):
                t = acc_pool.tile([PT, N], F32, tag=f"t{e - pe_e}")
                nc.scalar.activation(t[:], et3[:, :, e],
                                     mybir.ActivationFunctionType.Copy,
                                     scale=cb[:, e:e + 1])
                ts.append(t)
            s = acc_pool.tile([PT, N], F32, tag="s0")
            nc.vector.tensor_add(s[:], ts[0][:], ts[1][:])
            for k, t in enumerate(ts[2:]):
                s2 = acc_pool.tile([PT, N], F32, tag=f"s{k + 1}")
                nc.vector.tensor_add(s2[:], s[:], t[:])
                s = s2
            # DVE tensor_tensor must not mix PSUM+SBUF operands (HW fault):
            # copy the PE partial out of PSUM first, then add SBUF+SBUF.
            sc_sb = acc_pool.tile([PT, N], F32, tag="scsb")
            nc.vector.tensor_copy(sc_sb[:], sc_ps[:])
            sf = acc_pool.tile([PT, N], F32, tag="sf")
            nc.vector.tensor_add(sf[:], sc_sb[:], s[:])
            nc.scalar.activation(p_sb[:], sf[:],
                                 mybir.ActivationFunctionType.Exp, scale=1.0 / TAU)
        return p_sb

    def post_tile(b, it, p_sb, psup):
        """Transpose P, aggregate against pre_sup'+ones, normalize, relu, store."""
        pT_sb = pT_pool.tile([PT, N], F32)
        for jc in range(NIT):
            pT_ps = pT_psum.tile([PT, PT], F32)
            nc.tensor.matmul(pT_ps[:], p_sb[:, jc * PT:(jc + 1) * PT], idn[:],
                             is_transpose=True)
            nc.vector.tensor_copy(pT_sb[:, jc * PT:(jc + 1) * PT], pT_ps[:])
        o_ps = out_psum.tile([PT, D + 1], F32)
        for jc in range(NIT):
            nc.tensor.matmul(o_ps[:], pT_sb[:, jc * PT:(jc + 1) * PT],
                             psup[:, jc * (D + 1):(jc + 1) * (D + 1)],
                             start=(jc == 0), stop=(jc == NIT - 1))
        r = fin_pool.tile([PT, 1], F32, tag="r")
        nc.vector.reciprocal(r[:], o_ps[:, D:D + 1])
        t1 = fin_pool.tile([PT, D], F32, tag="t1")
        nc.vector.tensor_scalar_mul(t1[:], o_ps[:, 0:D], r[:])
        t2 = fin_pool.tile([PT, D], F32, tag="t2")
        nc.vector.tensor_add(t2[:], t1[:],
                             psup[:, it * (D + 1):it * (D + 1) + D])
        o_sb = o_pool.tile([PT, D], F32)
        nc.scalar.activation(o_sb[:], t2[:], mybir.ActivationFunctionType.Relu)
        row0 = b * N + it * PT
        nc.sync.dma_start(out[row0:row0 + PT, :], o_sb[:])

    # Software-pipelined emission: post(k-1) lands between scores(k) and
    # scores(k+1) so the PE never waits on ACT's exp.
    pending = None
    for b in range(BPC):
        psup = compute_psup(b)
        for it in range(NIT):
            p_sb = scores_tile(b, it)
            if pending is not None:
                post_tile(*pending)
            pending = (b, it, p_sb, psup)
    post_tile(*pending)


def _build_kernel_v5(ctx: ExitStack, tc: "tile.TileContext", edge, x, w, cdiag, ident, out,
                     coefb=None, identb=None, p_bf16: bool = True,
                     edge_bufs: int = 4):
    """v5: all 8 score e-terms as PE matmuls with the f32 edge tile BITCAST to
    f32r (1 cyc/row at free-dim 512 vs 4 for fp32 — no cast DMA needed, f32r
    is bit-identical). P/psup in bf16 so transposes and the aggregation
    matmuls also run 1 cyc/row and the DVE copy traffic halves. Finals in f32
    (skip-add uses the f32 psup). PE busy drops ~4x vs v3b; the kernel should
    sit on the HBM stream."""
    nc = tc.nc
    F32R = mybir.dt.float32r
    BF16 = mybir.dt.bfloat16
    PDT = BF16 if p_bf16 else F32

    consts = ctx.enter_context(tc.tile_pool(name="consts", bufs=1))
    edge_pool = ctx.enter_context(tc.tile_pool(name="edge", bufs=edge_bufs))
    xt_pool = ctx.enter_context(tc.tile_pool(name="xt", bufs=2))
    xT_pool = ctx.enter_context(tc.tile_pool(name="xT", bufs=2))
    psup_pool = ctx.enter_context(tc.tile_pool(name="psup", bufs=2))
    psupb_pool = ctx.enter_context(tc.tile_pool(name="psupb", bufs=2))
    p_pool = ctx.enter_context(tc.tile_pool(name="p", bufs=2))
    pT_pool = ctx.enter_context(tc.tile_pool(name="pT", bufs=2))
    fin_pool = ctx.enter_context(tc.tile_pool(name="fin", bufs=3))
    o_pool = ctx.enter_context(tc.tile_pool(name="o", bufs=3))

    misc_psum = ctx.enter_context(tc.tile_pool(name="mpsum", bufs=2, space="PSUM"))
    sc_psum = ctx.enter_context(tc.tile_pool(name="scpsum", bufs=2, space="PSUM"))
    pT_psum = ctx.enter_context(tc.tile_pool(name="ptpsum", bufs=2, space="PSUM"))
    out_psum = ctx.enter_context(tc.tile_pool(name="opsum", bufs=2, space="PSUM"))

    # Constants: coef-scaled identities as f32r (bit-identical to f32), a f32
    # identity for the x transposes, a PDT identity for the P transposes.
    cd = consts.tile([PT, E * PT], F32R)
    nc.sync.dma_start(cd[:], cdiag[:].bitcast(F32R))
    idn = consts.tile([PT, PT], F32)
    nc.sync.dma_start(idn[:], ident[:])
    if p_bf16:
        idp = consts.tile([PT, PT], BF16)
        nc.sync.dma_start(idp[:], identb[:])
    else:
        idp = idn
    wsb = consts.tile([D, D], F32)
    nc.sync.dma_start(wsb[:], w[:])

    x_r = x[:].rearrange("(b it p) d -> b p it d", b=BPC, it=NIT, p=PT)

    def compute_psup(b):
        """pre_sup' = 0.5 * (x_b @ W); f32 (for the skip-add) + a PDT copy
        with a trailing ones column per j-chunk (for the aggregation)."""
        xt = xt_pool.tile([PT, NIT * D], F32)
        nc.sync.dma_start(xt[:].rearrange("p (it d) -> p it d", it=NIT), x_r[b])
        psup = psup_pool.tile([PT, NIT * D], F32)
        psupb = psupb_pool.tile([PT, NIT * (D + 1)], PDT)
        for it in range(NIT):
            xT_ps = misc_psum.tile([D, PT], F32, tag="m")
            nc.tensor.matmul(xT_ps[:], xt[:, it * D:(it + 1) * D], idn[:],
                             is_transpose=True)
            xT_sb = xT_pool.tile([D, PT], F32)
            nc.vector.tensor_copy(xT_sb[:], xT_ps[:])
            ps_ps = misc_psum.tile([PT, D], F32, tag="m")
            nc.tensor.matmul(ps_ps[:], xT_sb[:], wsb[:], start=True, stop=True)
            nc.scalar.mul(psup[:, it * D:(it + 1) * D], ps_ps[:], 0.5)
            nc.vector.tensor_copy(psupb[:, it * (D + 1):it * (D + 1) + D],
                                  psup[:, it * D:(it + 1) * D])
            nc.vector.memset(psupb[:, it * (D + 1) + D:(it + 1) * (D + 1)], 1.0)
        return psup, psupb

    def scores_tile(b, it):
        """DMA one edge tile; 8 accumulating f32r score matmuls; exp -> PDT."""
        et = edge_pool.tile([PT, N * E], F32R)
        row0 = b * N + it * PT
        nc.sync.dma_start(et[:], edge[row0:row0 + PT, :].bitcast(F32R))
        et3 = et[:].rearrange("p (j e) -> p j e", e=E)
        sc_ps = sc_psum.tile([PT, N], F32)
        for e in range(E):
            nc.tensor.matmul(sc_ps[:], cd[:, e * PT:(e + 1) * PT], et3[:, :, e],
                             start=(e == 0), stop=(e == E - 1))
        p_sb = p_pool.tile([PT, N], PDT)
        nc.scalar.activation(p_sb[:], sc_ps[:],
                             mybir.ActivationFunctionType.Exp, scale=1.0 / TAU)
        return p_sb

    def post_tile(b, it, p_sb, psup, psupb):
        """Transpose P, aggregate against pre_sup'+ones, normalize, relu, store."""
        pT_sb = pT_pool.tile([PT, N], PDT)
        for jc in range(NIT):
            pT_ps = pT_psum.tile([PT, PT], PDT)
            nc.tensor.matmul(pT_ps[:], p_sb[:, jc * PT:(jc + 1) * PT], idp[:],
                             is_transpose=True)
            nc.vector.tensor_copy(pT_sb[:, jc * PT:(jc + 1) * PT], pT_ps[:])
        o_ps = out_psum.tile([PT, D + 1], F32)
        for jc in range(NIT):
            nc.tensor.matmul(o_ps[:], pT_sb[:, jc * PT:(jc + 1) * PT],
                             psupb[:, jc * (D + 1):(jc + 1) * (D + 1)],
                             start=(jc == 0), stop=(jc == NIT - 1))
        r = fin_pool.tile([PT, 1], F32, tag="r")
        nc.vector.reciprocal(r[:], o_ps[:, D:D + 1])
        t1 = fin_pool.tile([PT, D], F32, tag="t1")
        nc.vector.tensor_scalar_mul(t1[:], o_ps[:, 0:D], r[:])
        t2 = fin_pool.tile([PT, D], F32, tag="t2")
        nc.vector.tensor_add(t2[:], t1[:], psup[:, it * D:(it + 1) * D])
        o_sb = o_pool.tile([PT, D], F32)
        nc.scalar.activation(o_sb[:], t2[:], mybir.ActivationFunctionType.Relu)
        row0 = b * N + it * PT
        nc.sync.dma_start(out[row0:row0 + PT, :], o_sb[:])

    pending = None
    for b in range(BPC):
        psup, psupb = compute_psup(b)
        for it in range(NIT):
            p_sb = scores_tile(b, it)
            if pending is not None:
                post_tile(*pending)
            pending = (b, it, p_sb, psup, psupb)
    post_tile(*pending)


def _build_dma_only(ctx: ExitStack, tc: "tile.TileContext", edge, x, w, cdiag, ident, out,
                    coefb=None, identb=None):
    """Variant: just the edge DMA stream + a trivial out write (BW probe)."""
    nc = tc.nc
    edge_pool = ctx.enter_context(tc.tile_pool(name="edge", bufs=4))
    o_pool = ctx.enter_context(tc.tile_pool(name="o", bufs=2))
    for b in range(BPC):
        for it in range(NIT):
            et = edge_pool.tile([PT, N * E], F32)
            row0 = b * N + it * PT
            nc.sync.dma_start(et[:], edge[row0:row0 + PT, :])
            o_sb = o_pool.tile([PT, D], F32)
            nc.vector.tensor_copy(o_sb[:], et[:, 0:D])
            nc.sync.dma_start(out[row0:row0 + PT, :], o_sb[:])


def _build_kernel_f32r(ctx, tc, edge, x, w, cdiag, ident, out, coefb=None, identb=None):
    _build_kernel(ctx, tc, edge, x, w, cdiag, ident, out, scores_f32r=True)


def _build_kernel_split4(ctx, tc, edge, x, w, cdiag, ident, out, coefb=None, identb=None):
    _build_kernel(ctx, tc, edge, x, w, cdiag, ident, out, pe_e=4, coefb=coefb)


def _build_kernel_split5(ctx, tc, edge, x, w, cdiag, ident, out, coefb=None, identb=None):
    _build_kernel(ctx, tc, edge, x, w, cdiag, ident, out, pe_e=5, coefb=coefb)


def _build_kernel_v4(ctx, tc, edge, x, w, cdiag, ident, out, coefb=None, identb=None):
    _build_kernel(ctx, tc, edge, x, w, cdiag, ident, out, pe_e=5, coefb=coefb,
                  edge_bufs=6)


def _build_kernel_v5f(ctx, tc, edge, x, w, cdiag, ident, out, coefb=None, identb=None):
    _build_kernel_v5(ctx, tc, edge, x, w, cdiag, ident, out, identb=identb,
                     p_bf16=False)


_BUILDERS = {"v1": _build_kernel, "v2": _build_kernel_f32r, "dma": _build_dma_only,
             "v3": _build_kernel_split4, "v3b": _build_kernel_split5,
             "v4": _build_kernel_v4, "v5": _build_kernel_v5,
             "v5f": _build_kernel_v5f}


def _get_nc(reps: int = 1, variant: str = "v1"):
    key = f"{variant}-r{reps}"
    internal_edge = variant.endswith("i")
    base_variant = variant[:-1] if internal_edge else variant
    if key not in _nc_cache:
        nc = bacc.Bacc("TRN2", target_bir_lowering=False, debug=False,
                       num_devices=NCORES)
        if internal_edge:
            # Bench-only: edge lives in device DRAM (uninitialized) so the
            # axon tunnel doesn't re-ship 512 MiB per timed call.
            edge = nc.dram_tensor("edge_int", [BPC * N, N * E], F32)
        else:
            edge = nc.declare_dram_parameter("edge", [BPC * N, N * E], F32, isOutput=False)
        x = nc.declare_dram_parameter("x", [BPC * N, D], F32, isOutput=False)
        w = nc.declare_dram_parameter("w", [D, D], F32, isOutput=False)
        cdiag = nc.declare_dram_parameter("cdiag", [PT, E * PT], F32, isOutput=False)
        ident = nc.declare_dram_parameter("ident", [PT, PT], F32, isOutput=False)
        coefb = nc.declare_dram_parameter("coefb", [PT, E], F32, isOutput=False)
        identb = nc.declare_dram_parameter("identb", [PT, PT], mybir.dt.bfloat16,
                                           isOutput=False)
        out = nc.declare_dram_parameter("out", [BPC * N, D], F32, isOutput=True)
        builder = _BUILDERS[base_variant]
        with tile.TileContext(nc) as tc:
            for _ in range(reps):
                with ExitStack() as ctx:
                    builder(ctx, tc, edge, x, w, cdiag, ident, out, coefb=coefb,
                            identb=identb)
        nc.compile()
        _nc_cache[key] = nc
    return _nc_cache[key]


def kernel(**inputs) -> np.ndarray:
    global LAST_RESULT
    edge = np.ascontiguousarray(inputs["edge_features"], dtype=np.float32)
    x = np.ascontiguousarray(inputs["x"], dtype=np.float32)
    W = np.ascontiguousarray(inputs["W"], dtype=np.float32)
    coef = np.asarray(inputs["coef"], dtype=np.float32)

    c = coef[:, 0]
    cdiag = np.zeros((PT, E * PT), np.float32)
    ar = np.arange(PT)
    for e in range(E):
        cdiag[ar, e * PT + ar] = c[e]
    ident = np.eye(PT, dtype=np.float32)

    nc = _get_nc(variant=VARIANT)
    in_maps = []
    for core in range(NCORES):
        b0 = core * BPC
        in_maps.append({
            "edge": edge[b0:b0 + BPC].reshape(BPC * N, N * E),
            "x": x[b0 * N:(b0 + BPC) * N],
            "w": W,
            "cdiag": cdiag,
            "ident": ident,
            "coefb": np.repeat(c[None, :], PT, axis=0),
            "identb": np.eye(PT, dtype=ml_dtypes.bfloat16),
        })
    res = run_bass_kernel_spmd(nc, in_maps, list(range(NCORES)), trace=TRACE)
    LAST_RESULT = res
    return np.concatenate([res.results[i]["out"] for i in range(NCORES)], axis=0)



# revision 11
# speedup vs baseline: 1.4766x; 1.0134x over previous
"""GNN message-passing layer (GCN w/ edge-feature attention) on 8 trn2 cores.

Math (per graph b, N=512 nodes, E=8 edge feats, D=64):
    pre_sup = x_b @ W                                   [N, D]
    s[i,j]  = sum_e coef[e] * edge[b,i,j,e]             [N, N]
    adj     = softmax(s / tau, axis=-1)   (tau = 1.0)
    adj_hat = adj + I;  d = rowsum(adj_hat) = 2 exactly (softmax rows sum to 1)
    out     = relu(0.5 * adj_hat @ pre_sup)
            = relu( (P @ (0.5*pre_sup)) / Z + 0.5*pre_sup )
  where P = exp(s) (unnormalized, no max-subtraction needed: |s| <~ 25),
  Z_i = sum_j P[i,j] obtained for free as an extra ones-column in the
  aggregation matmul.

Device mapping (per core: 8 graphs, 64 MiB of edge data = the roofline):
  - scores (default v3b): 5 of the 8 e-terms as PSUM-accumulated PE matmuls
    (lhsT = coef[e]*I_128, rhs = stride-8 e-slice; fp32 matmul is 4 cyc/row,
    so splitting engines beats PE-only); the other 3 e-terms as ACT
    scaled-copies (per-partition scale AP = coef[e]) + DVE tree adds.
    NB: DVE tensor_tensor must not mix PSUM+SBUF operands (HW fault) -- the
    PE partial is tensor_copy'd out of PSUM before the final add.
  - exp: ACT engine (no max-subtraction needed; |scores| < ~25).
  - transpose P tiles on PE (is_transpose matmul vs identity), copy to SBUF
    on DVE, then aggregation matmuls contract j with rhs=[0.5*pre_sup | 1];
    the ones column yields the softmax denominator Z for free.
  - finals: reciprocal + per-partition scale + skip add + relu, DMA out.
  Cost-model (TimelineSim): ~238 us/iter, DMA-bound (DMA 200, PE 172,
  DVE 130, ACT 95 us busy); v1 (PE-only scores) was 269 us, PE-bound.
"""

import os
from contextlib import ExitStack

import ml_dtypes
import numpy as np

import concourse.bass as bass
import concourse.tile as tile
from concourse import bacc, mybir
from concourse.bass_utils import run_bass_kernel_spmd

F32 = mybir.dt.float32

B, N, E, D = 64, 512, 8, 64
NCORES = 8
BPC = B // NCORES          # graphs per core
PT = 128                   # partition tile (i-rows per edge tile)
NIT = N // PT              # 4 i-tiles (and j-chunks) per graph
TAU = 1.0

# Module-level knobs (test.py pokes these)
TRACE = os.environ.get("KERNEL_TRACE", "") == "1"
VARIANT = os.environ.get("KERNEL_VARIANT", "v3b")
LAST_RESULT = None

_nc_cache = {}


def _build_kernel(ctx: ExitStack, tc: "tile.TileContext", edge, x, w, cdiag, ident, out,
                  scores_f32r: bool = False, pe_e: int = E, coefb=None,
                  identb=None, edge_bufs: int = 4):
    """pe_e: how many of the E per-edge-feature score terms run as PE matmuls;
    the remaining E-pe_e run as ACT scaled-copies + DVE tree adds (fp32)."""
    nc = tc.nc
    EDT = mybir.dt.float32r if scores_f32r else F32

    consts = ctx.enter_context(tc.tile_pool(name="consts", bufs=1))
    if pe_e < E:
        acc_pool = ctx.enter_context(tc.tile_pool(name="acc", bufs=2))
        cb = consts.tile([PT, E], F32)
        nc.sync.dma_start(cb[:], coefb[:])
    edge_pool = ctx.enter_context(tc.tile_pool(name="edge", bufs=edge_bufs))
    xt_pool = ctx.enter_context(tc.tile_pool(name="xt", bufs=2))
    xT_pool = ctx.enter_context(tc.tile_pool(name="xT", bufs=2))
    psup_pool = ctx.enter_context(tc.tile_pool(name="psup", bufs=2))
    p_pool = ctx.enter_context(tc.tile_pool(name="p", bufs=2))
    pT_pool = ctx.enter_context(tc.tile_pool(name="pT", bufs=2))
    fin_pool = ctx.enter_context(tc.tile_pool(name="fin", bufs=3))
    o_pool = ctx.enter_context(tc.tile_pool(name="o", bufs=3))

    misc_psum = ctx.enter_context(tc.tile_pool(name="mpsum", bufs=2, space="PSUM"))
    sc_psum = ctx.enter_context(tc.tile_pool(name="scpsum", bufs=2, space="PSUM"))
    pT_psum = ctx.enter_context(tc.tile_pool(name="ptpsum", bufs=2, space="PSUM"))
    out_psum = ctx.enter_context(tc.tile_pool(name="opsum", bufs=2, space="PSUM"))

    # Constants
    cd = consts.tile([PT, E * PT], EDT)       # cd[:, e*128:(e+1)*128] = coef[e] * I
    if scores_f32r:
        nc.gpsimd.dma_start(cd[:], cdiag[:])  # SWDGE casts f32 -> f32r inline
    else:
        nc.sync.dma_start(cd[:], cdiag[:])
    idn = consts.tile([PT, PT], F32)
    nc.sync.dma_start(idn[:], ident[:])
    wsb = consts.tile([D, D], F32)
    nc.sync.dma_start(wsb[:], w[:])

    x_r = x[:].rearrange("(b it p) d -> b p it d", b=BPC, it=NIT, p=PT)

    def compute_psup(b):
        """pre_sup' = 0.5 * (x_b @ W) with a trailing ones column per j-chunk."""
        xt = xt_pool.tile([PT, NIT * D], F32)
        nc.sync.dma_start(xt[:].rearrange("p (it d) -> p it d", it=NIT), x_r[b])
        psup = psup_pool.tile([PT, NIT * (D + 1)], F32)
        for it in range(NIT):
            xT_ps = misc_psum.tile([D, PT], F32, tag="m")
            nc.tensor.matmul(xT_ps[:], xt[:, it * D:(it + 1) * D], idn[:],
                             is_transpose=True)
            xT_sb = xT_pool.tile([D, PT], F32)
            nc.vector.tensor_copy(xT_sb[:], xT_ps[:])
            ps_ps = misc_psum.tile([PT, D], F32, tag="m")
            nc.tensor.matmul(ps_ps[:], xT_sb[:], wsb[:], start=True, stop=True)
            nc.scalar.mul(psup[:, it * (D + 1):it * (D + 1) + D], ps_ps[:], 0.5)
            nc.vector.memset(psup[:, it * (D + 1) + D:(it + 1) * (D + 1)], 1.0)
        return psup

    def scores_tile(b, it):
        """DMA one edge tile and run the 8 accumulating score matmuls."""
        et = edge_pool.tile([PT, N * E], EDT)
        row0 = b * N + it * PT
        if scores_f32r:
            nc.gpsimd.dma_start(et[:], edge[row0:row0 + PT, :])
        else:
            nc.sync.dma_start(et[:], edge[row0:row0 + PT, :])
        et3 = et[:].rearrange("p (j e) -> p j e", e=E)
        sc_ps = sc_psum.tile([PT, N], F32)
        for e in range(pe_e):
            nc.tensor.matmul(sc_ps[:], cd[:, e * PT:(e + 1) * PT], et3[:, :, e],
                             start=(e == 0), stop=(e == pe_e - 1))
        p_sb = p_pool.tile([PT, N], F32)
        if pe_e == E:
            nc.scalar.activation(p_sb[:], sc_ps[:],
                                 mybir.ActivationFunctionType.Exp, scale=1.0 / TAU)
        else:
            # ACT: t_e = coef[e] * edge[:, :, e]; DVE: tree-add + fold in PSUM.
            ts = []
            for e in range(pe_e, E):
                t = acc_pool.tile([PT, N], F32, tag=f"t{e - pe_e}")
                nc.scalar.activation(t[:], et3[:, :, e],
                                     mybir.ActivationFunctionType.Copy,
                                     scale=cb[:, e:e + 1])
                ts.append(t)
            s = acc_pool.tile([PT, N], F32, tag="s0")
            nc.vector.tensor_add(s[:], ts[0][:], ts[1][:])
            for k, t in enumerate(ts[2:]):
                s2 = acc_pool.tile([PT, N], F32, tag=f"s{k + 1}")
                nc.vector.tensor_add(s2[:], s[:], t[:])
                s = s2
            # DVE tensor_tensor must not mix PSUM+SBUF operands (HW fault):
            # copy the PE partial out of PSUM first, then add SBUF+SBUF.
            sc_sb = acc_pool.tile([PT, N], F32, tag="scsb")
            nc.vector.tensor_copy(sc_sb[:], sc_ps[:])
            sf = acc_pool.tile([PT, N], F32, tag="sf")
            nc.vector.tensor_add(sf[:], sc_sb[:], s[:])
            nc.scalar.activation(p_sb[:], sf[:],
                                 mybir.ActivationFunctionType.Exp, scale=1.0 / TAU)
        return p_sb

    def post_tile(b, it, p_sb, psup):
        """Transpose P, aggregate against pre_sup'+ones, normalize, relu, store."""
        pT_sb = pT_pool.tile([PT, N], F32)
        for jc in range(NIT):
            pT_ps = pT_psum.tile([PT, PT], F32)
            nc.tensor.matmul(pT_ps[:], p_sb[:, jc * PT:(jc + 1) * PT], idn[:],
                             is_transpose=True)
            nc.vector.tensor_copy(pT_sb[:, jc * PT:(jc + 1) * PT], pT_ps[:])
        o_ps = out_psum.tile([PT, D + 1], F32)
        for jc in range(NIT):
            nc.tensor.matmul(o_ps[:], pT_sb[:, jc * PT:(jc + 1) * PT],
                             psup[:, jc * (D + 1):(jc + 1) * (D + 1)],
                             start=(jc == 0), stop=(jc == NIT - 1))
        r = fin_pool.tile([PT, 1], F32, tag="r")
        nc.vector.reciprocal(r[:], o_ps[:, D:D + 1])
        t1 = fin_pool.tile([PT, D], F32, tag="t1")
        nc.vector.tensor_scalar_mul(t1[:], o_ps[:, 0:D], r[:])
        t2 = fin_pool.tile([PT, D], F32, tag="t2")
        nc.vector.tensor_add(t2[:], t1[:],
                             psup[:, it * (D + 1):it * (D + 1) + D])
        o_sb = o_pool.tile([PT, D], F32)
        nc.scalar.activation(o_sb[:], t2[:], mybir.ActivationFunctionType.Relu)
        row0 = b * N + it * PT
        nc.sync.dma_start(out[row0:row0 + PT, :], o_sb[:])

    # Software-pipelined emission: post(k-1) lands between scores(k) and
    # scores(k+1) so the PE never waits on ACT's exp.
    pending = None
    for b in range(BPC):
        psup = compute_psup(b)
        for it in range(NIT):
            p_sb = scores_tile(b, it)
            if pending is not None:
                post_tile(*pending)
            pending = (b, it, p_sb, psup)
    post_tile(*pending)


def _build_kernel_v5(ctx: ExitStack, tc: "tile.TileContext", edge, x, w, cdiag, ident, out,
                     coefb=None, identb=None, p_bf16: bool = True,
                     edge_bufs: int = 4):
    """v5: all 8 score e-terms as PE matmuls with the f32 edge tile BITCAST to
    f32r (1 cyc/row at free-dim 512 vs 4 for fp32 — no cast DMA needed, f32r
    is bit-identical). P/psup in bf16 so transposes and the aggregation
    matmuls also run 1 cyc/row and the DVE copy traffic halves. Finals in f32
    (skip-add uses the f32 psup). PE busy drops ~4x vs v3b; the kernel should
    sit on the HBM stream."""
    nc = tc.nc
    F32R = mybir.dt.float32r
    BF16 = mybir.dt.bfloat16
    PDT = BF16 if p_bf16 else F32

    consts = ctx.enter_context(tc.tile_pool(name="consts", bufs=1))
    edge_pool = ctx.enter_context(tc.tile_pool(name="edge", bufs=edge_bufs))
    xt_pool = ctx.enter_context(tc.tile_pool(name="xt", bufs=2))
    xT_pool = ctx.enter_context(tc.tile_pool(name="xT", bufs=2))
    psup_pool = ctx.enter_context(tc.tile_pool(name="psup", bufs=2))
    psupb_pool = ctx.enter_context(tc.tile_pool(name="psupb", bufs=2))
    p_pool = ctx.enter_context(tc.tile_pool(name="p", bufs=2))
    pT_pool = ctx.enter_context(tc.tile_pool(name="pT", bufs=2))
    fin_pool = ctx.enter_context(tc.tile_pool(name="fin", bufs=3))
    o_pool = ctx.enter_context(tc.tile_pool(name="o", bufs=3))

    misc_psum = ctx.enter_context(tc.tile_pool(name="mpsum", bufs=2, space="PSUM"))
    sc_psum = ctx.enter_context(tc.tile_pool(name="scpsum", bufs=2, space="PSUM"))
    pT_psum = ctx.enter_context(tc.tile_pool(name="ptpsum", bufs=2, space="PSUM"))
    out_psum = ctx.enter_context(tc.tile_pool(name="opsum", bufs=2, space="PSUM"))

    # Constants: coef-scaled identities as f32r (bit-identical to f32), a f32
    # identity for the x transposes, a PDT identity for the P transposes.
    cd = consts.tile([PT, E * PT], F32R)
    nc.sync.dma_start(cd[:], cdiag[:].bitcast(F32R))
    idn = consts.tile([PT, PT], F32)
    nc.sync.dma_start(idn[:], ident[:])
    if p_bf16:
        idp = consts.tile([PT, PT], BF16)
        nc.sync.dma_start(idp[:], identb[:])
    else:
        idp = idn
    wsb = consts.tile([D, D], F32)
    nc.sync.dma_start(wsb[:], w[:])

    x_r = x[:].rearrange("(b it p) d -> b p it d", b=BPC, it=NIT, p=PT)

    def compute_psup(b):
        """pre_sup' = 0.5 * (x_b @ W); f32 (for the skip-add) + a PDT copy
        with a trailing ones column per j-chunk (for the aggregation)."""
        xt = xt_pool.tile([PT, NIT * D], F32)
        nc.sync.dma_start(xt[:].rearrange("p (it d) -> p it d", it=NIT), x_r[b])
        psup = psup_pool.tile([PT, NIT * D], F32)
        psupb = psupb_pool.tile([PT, NIT * (D + 1)], PDT)
        for it in range(NIT):
            xT_ps = misc_psum.tile([D, PT], F32, tag="m")
            nc.tensor.matmul(xT_ps[:], xt[:, it * D:(it + 1) * D], idn[:],
                             is_transpose=True)
            xT_sb = xT_pool.tile([D, PT], F32)
            nc.vector.tensor_copy(xT_sb[:], xT_ps[:])
            ps_ps = misc_psum.tile([PT, D], F32, tag="m")
            nc.tensor.matmul(ps_ps[:], xT_sb[:], wsb[:], start=True, stop=True)
            nc.scalar.mul(psup[:, it * D:(it + 1) * D], ps_ps[:], 0.5)
            nc.vector.tensor_copy(psupb[:, it * (D + 1):it * (D + 1) + D],
                                  psup[:, it * D:(it + 1) * D])
            nc.vector.memset(psupb[:, it * (D + 1) + D:(it + 1) * (D + 1)], 1.0)
        return psup, psupb

    def scores_tile(b, it):
        """DMA one edge tile; 8 accumulating f32r score matmuls; exp -> PDT."""
        et = edge_pool.tile([PT, N * E], F32R)
        row0 = b * N + it * PT
        nc.sync.dma_start(et[:], edge[row0:row0 + PT, :].bitcast(F32R))
        et3 = et[:].rearrange("p (j e) -> p j e", e=E)
        sc_ps = sc_psum.tile([PT, N], F32)
        for e in range(E):
            nc.tensor.matmul(sc_ps[:], cd[:, e * PT:(e + 1) * PT], et3[:, :, e],
                             start=(e == 0), stop=(e == E - 1))
        p_sb = p_pool.tile([PT, N], PDT)
        nc.scalar.activation(p_sb[:], sc_ps[:],
                             mybir.ActivationFunctionType.Exp, scale=1.0 / TAU)
        return p_sb

    def post_tile(b, it, p_sb, psup, psupb):
        """Transpose P, aggregate against pre_sup'+ones, normalize, relu, store."""
        pT_sb = pT_pool.tile([PT, N], PDT)
        for jc in range(NIT):
            pT_ps = pT_psum.tile([PT, PT], PDT)
            nc.tensor.matmul(pT_ps[:], p_sb[:, jc * PT:(jc + 1) * PT], idp[:],
                             is_transpose=True)
            nc.vector.tensor_copy(pT_sb[:, jc * PT:(jc + 1) * PT], pT_ps[:])
        o_ps = out_psum.tile([PT, D + 1], F32)
        for jc in range(NIT):
            nc.tensor.matmul(o_ps[:], pT_sb[:, jc * PT:(jc + 1) * PT],
                             psupb[:, jc * (D + 1):(jc + 1) * (D + 1)],
                             start=(jc == 0), stop=(jc == NIT - 1))
        r = fin_pool.tile([PT, 1], F32, tag="r")
        nc.vector.reciprocal(r[:], o_ps[:, D:D + 1])
        t1 = fin_pool.tile([PT, D], F32, tag="t1")
        nc.vector.tensor_scalar_mul(t1[:], o_ps[:, 0:D], r[:])
        t2 = fin_pool.tile([PT, D], F32, tag="t2")
        nc.vector.tensor_add(t2[:], t1[:], psup[:, it * D:(it + 1) * D])
        o_sb = o_pool.tile([PT, D], F32)
        nc.scalar.activation(o_sb[:], t2[:], mybir.ActivationFunctionType.Relu)
        row0 = b * N + it * PT
        nc.sync.dma_start(out[row0:row0 + PT, :], o_sb[:])

    pending = None
    for b in range(BPC):
        psup, psupb = compute_psup(b)
        for it in range(NIT):
            p_sb = scores_tile(b, it)
            if pending is not None:
                post_tile(*pending)
            pending = (b, it, p_sb, psup, psupb)
    post_tile(*pending)


def _build_kernel_v6(ctx: ExitStack, tc: "tile.TileContext", edge, x, w, cdiag, ident, out,
                     coefb=None, identb=None, hsplit: int = 2, edge_bufs: int = 8):
    """v6 = v5 with each 2 MiB edge tile split into `hsplit` j-range chunks.
    Finer DMA/compute interleave: PE gets work every ~DMA-chunk instead of
    bursting once per 2 MiB tile, which keeps the HAM clock-gate warm (v5
    measured 210 us throttled at K=4/8) and shortens the pipeline tail."""
    nc = tc.nc
    F32R = mybir.dt.float32r
    BF16 = mybir.dt.bfloat16
    PDT = BF16
    JW = N // hsplit              # j-width per chunk (>=256 keeps f32r 1 cyc/row)
    JB = JW // PT                 # 128-blocks per chunk

    consts = ctx.enter_context(tc.tile_pool(name="consts", bufs=1))
    edge_pool = ctx.enter_context(tc.tile_pool(name="edge", bufs=edge_bufs))
    xt_pool = ctx.enter_context(tc.tile_pool(name="xt", bufs=2))
    xT_pool = ctx.enter_context(tc.tile_pool(name="xT", bufs=2))
    psup_pool = ctx.enter_context(tc.tile_pool(name="psup", bufs=2))
    psupb_pool = ctx.enter_context(tc.tile_pool(name="psupb", bufs=2))
    p_pool = ctx.enter_context(tc.tile_pool(name="p", bufs=3))
    pT_pool = ctx.enter_context(tc.tile_pool(name="pT", bufs=3))
    fin_pool = ctx.enter_context(tc.tile_pool(name="fin", bufs=3))
    o_pool = ctx.enter_context(tc.tile_pool(name="o", bufs=3))

    misc_psum = ctx.enter_context(tc.tile_pool(name="mpsum", bufs=2, space="PSUM"))
    sc_psum = ctx.enter_context(tc.tile_pool(name="scpsum", bufs=2, space="PSUM"))
    pT_psum = ctx.enter_context(tc.tile_pool(name="ptpsum", bufs=2, space="PSUM"))
    out_psum = ctx.enter_context(tc.tile_pool(name="opsum", bufs=2, space="PSUM"))

    cd = consts.tile([PT, E * PT], F32R)
    nc.sync.dma_start(cd[:], cdiag[:].bitcast(F32R))
    idn = consts.tile([PT, PT], F32)
    nc.sync.dma_start(idn[:], ident[:])
    idp = consts.tile([PT, PT], BF16)
    nc.sync.dma_start(idp[:], identb[:])
    wsb = consts.tile([D, D], F32)
    nc.sync.dma_start(wsb[:], w[:])

    x_r = x[:].rearrange("(b it p) d -> b p it d", b=BPC, it=NIT, p=PT)

    def compute_psup(b):
        xt = xt_pool.tile([PT, NIT * D], F32)
        nc.sync.dma_start(xt[:].rearrange("p (it d) -> p it d", it=NIT), x_r[b])
        psup = psup_pool.tile([PT, NIT * D], F32)
        psupb = psupb_pool.tile([PT, NIT * (D + 1)], PDT)
        for it in range(NIT):
            xT_ps = misc_psum.tile([D, PT], F32, tag="m")
            nc.tensor.matmul(xT_ps[:], xt[:, it * D:(it + 1) * D], idn[:],
                             is_transpose=True)
            xT_sb = xT_pool.tile([D, PT], F32)
            nc.vector.tensor_copy(xT_sb[:], xT_ps[:])
            ps_ps = misc_psum.tile([PT, D], F32, tag="m")
            nc.tensor.matmul(ps_ps[:], xT_sb[:], wsb[:], start=True, stop=True)
            nc.scalar.mul(psup[:, it * D:(it + 1) * D], ps_ps[:], 0.5)
            nc.vector.tensor_copy(psupb[:, it * (D + 1):it * (D + 1) + D],
                                  psup[:, it * D:(it + 1) * D])
            nc.vector.memset(psupb[:, it * (D + 1) + D:(it + 1) * (D + 1)], 1.0)
        return psup, psupb

    def scores_chunk(b, it, h):
        """DMA one [PT, JW*E] edge chunk; 8 accumulating f32r matmuls; exp."""
        et = edge_pool.tile([PT, JW * E], F32R)
        row0 = b * N + it * PT
        nc.sync.dma_start(et[:],
                          edge[row0:row0 + PT, h * JW * E:(h + 1) * JW * E]
                          .bitcast(F32R))
        et3 = et[:].rearrange("p (j e) -> p j e", e=E)
        sc_ps = sc_psum.tile([PT, JW], F32)
        for e in range(E):
            nc.tensor.matmul(sc_ps[:], cd[:, e * PT:(e + 1) * PT], et3[:, :, e],
                             start=(e == 0), stop=(e == E - 1))
        p_sb = p_pool.tile([PT, JW], PDT)
        nc.scalar.activation(p_sb[:], sc_ps[:],
                             mybir.ActivationFunctionType.Exp, scale=1.0 / TAU)
        return p_sb

    def post_chunk(b, it, h, p_sb, psupb, o_ps):
        """Transpose this chunk's P blocks and fold them into o_ps."""
        pT_sb = pT_pool.tile([PT, JW], PDT)
        for jj in range(JB):
            pT_ps = pT_psum.tile([PT, PT], PDT)
            nc.tensor.matmul(pT_ps[:], p_sb[:, jj * PT:(jj + 1) * PT], idp[:],
                             is_transpose=True)
            nc.vector.tensor_copy(pT_sb[:, jj * PT:(jj + 1) * PT], pT_ps[:])
        for jj in range(JB):
            jc = h * JB + jj
            nc.tensor.matmul(o_ps[:], pT_sb[:, jj * PT:(jj + 1) * PT],
                             psupb[:, jc * (D + 1):(jc + 1) * (D + 1)],
                             start=(jc == 0), stop=(jc == NIT - 1))

    def finals(b, it, psup, o_ps):
        r = fin_pool.tile([PT, 1], F32, tag="r")
        nc.vector.reciprocal(r[:], o_ps[:, D:D + 1])
        t1 = fin_pool.tile([PT, D], F32, tag="t1")
        nc.vector.tensor_scalar_mul(t1[:], o_ps[:, 0:D], r[:])
        t2 = fin_pool.tile([PT, D], F32, tag="t2")
        nc.vector.tensor_add(t2[:], t1[:], psup[:, it * D:(it + 1) * D])
        o_sb = o_pool.tile([PT, D], F32)
        nc.scalar.activation(o_sb[:], t2[:], mybir.ActivationFunctionType.Relu)
        row0 = b * N + it * PT
        nc.sync.dma_start(out[row0:row0 + PT, :], o_sb[:])

    pending = None       # (b, it, h, p_sb, psupb, o_ps)
    fin_pending = None   # (b, it, psup, o_ps)
    for b in range(BPC):
        psup, psupb = compute_psup(b)
        for it in range(NIT):
            o_ps = out_psum.tile([PT, D + 1], F32)
            for h in range(hsplit):
                p_sb = scores_chunk(b, it, h)
                if pending is not None:
                    post_chunk(*pending)
                    if pending[2] == hsplit - 1:
                        finals(*fin_pending)
                pending = (b, it, h, p_sb, psupb, o_ps)
                if h == hsplit - 1:
                    fin_pending = (b, it, psup, o_ps)
    post_chunk(*pending)
    finals(*fin_pending)


def _build_dma_only(ctx: ExitStack, tc: "tile.TileContext", edge, x, w, cdiag, ident, out,
                    coefb=None, identb=None):
    """Variant: just the edge DMA stream + a trivial out write (BW probe)."""
    nc = tc.nc
    edge_pool = ctx.enter_context(tc.tile_pool(name="edge", bufs=4))
    o_pool = ctx.enter_context(tc.tile_pool(name="o", bufs=2))
    for b in range(BPC):
        for it in range(NIT):
            et = edge_pool.tile([PT, N * E], F32)
            row0 = b * N + it * PT
            nc.sync.dma_start(et[:], edge[row0:row0 + PT, :])
            o_sb = o_pool.tile([PT, D], F32)
            nc.vector.tensor_copy(o_sb[:], et[:, 0:D])
            nc.sync.dma_start(out[row0:row0 + PT, :], o_sb[:])


def _build_kernel_f32r(ctx, tc, edge, x, w, cdiag, ident, out, coefb=None, identb=None):
    _build_kernel(ctx, tc, edge, x, w, cdiag, ident, out, scores_f32r=True)


def _build_kernel_split4(ctx, tc, edge, x, w, cdiag, ident, out, coefb=None, identb=None):
    _build_kernel(ctx, tc, edge, x, w, cdiag, ident, out, pe_e=4, coefb=coefb)


def _build_kernel_split5(ctx, tc, edge, x, w, cdiag, ident, out, coefb=None, identb=None):
    _build_kernel(ctx, tc, edge, x, w, cdiag, ident, out, pe_e=5, coefb=coefb)


def _build_kernel_v4(ctx, tc, edge, x, w, cdiag, ident, out, coefb=None, identb=None):
    _build_kernel(ctx, tc, edge, x, w, cdiag, ident, out, pe_e=5, coefb=coefb,
                  edge_bufs=6)


def _build_kernel_v5f(ctx, tc, edge, x, w, cdiag, ident, out, coefb=None, identb=None):
    _build_kernel_v5(ctx, tc, edge, x, w, cdiag, ident, out, identb=identb,
                     p_bf16=False)


def _build_kernel_v6b(ctx, tc, edge, x, w, cdiag, ident, out, coefb=None, identb=None):
    _build_kernel_v6(ctx, tc, edge, x, w, cdiag, ident, out, identb=identb,
                     hsplit=2, edge_bufs=12)


_BUILDERS = {"v1": _build_kernel, "v2": _build_kernel_f32r, "dma": _build_dma_only,
             "v3": _build_kernel_split4, "v3b": _build_kernel_split5,
             "v4": _build_kernel_v4, "v5": _build_kernel_v5,
             "v5f": _build_kernel_v5f, "v6": _build_kernel_v6,
             "v6b": _build_kernel_v6b}


def _get_nc(reps: int = 1, variant: str = "v1"):
    key = f"{variant}-r{reps}"
    internal_edge = variant.endswith("i")
    base_variant = variant[:-1] if internal_edge else variant
    if key not in _nc_cache:
        nc = bacc.Bacc("TRN2", target_bir_lowering=False, debug=False,
                       num_devices=NCORES)
        if internal_edge:
            # Bench-only: edge lives in device DRAM (uninitialized) so the
            # axon tunnel doesn't re-ship 512 MiB per timed call.
            edge = nc.dram_tensor("edge_int", [BPC * N, N * E], F32)
        else:
            edge = nc.declare_dram_parameter("edge", [BPC * N, N * E], F32, isOutput=False)
        x = nc.declare_dram_parameter("x", [BPC * N, D], F32, isOutput=False)
        w = nc.declare_dram_parameter("w", [D, D], F32, isOutput=False)
        cdiag = nc.declare_dram_parameter("cdiag", [PT, E * PT], F32, isOutput=False)
        ident = nc.declare_dram_parameter("ident", [PT, PT], F32, isOutput=False)
        coefb = nc.declare_dram_parameter("coefb", [PT, E], F32, isOutput=False)
        identb = nc.declare_dram_parameter("identb", [PT, PT], mybir.dt.bfloat16,
                                           isOutput=False)
        out = nc.declare_dram_parameter("out", [BPC * N, D], F32, isOutput=True)
        builder = _BUILDERS[base_variant]
        with tile.TileContext(nc) as tc:
            for _ in range(reps):
                with ExitStack() as ctx:
                    builder(ctx, tc, edge, x, w, cdiag, ident, out, coefb=coefb,
                            identb=identb)
        nc.compile()
        _nc_cache[key] = nc
    return _nc_cache[key]


def kernel(**inputs) -> np.ndarray:
    global LAST_RESULT
    edge = np.ascontiguousarray(inputs["edge_features"], dtype=np.float32)
    x = np.ascontiguousarray(inputs["x"], dtype=np.float32)
    W = np.ascontiguousarray(inputs["W"], dtype=np.float32)
    coef = np.asarray(inputs["coef"], dtype=np.float32)

    c = coef[:, 0]
    cdiag = np.zeros((PT, E * PT), np.float32)
    ar = np.arange(PT)
    for e in range(E):
        cdiag[ar, e * PT + ar] = c[e]
    ident = np.eye(PT, dtype=np.float32)

    nc = _get_nc(variant=VARIANT)
    in_maps = []
    for core in range(NCORES):
        b0 = core * BPC
        in_maps.append({
            "edge": edge[b0:b0 + BPC].reshape(BPC * N, N * E),
            "x": x[b0 * N:(b0 + BPC) * N],
            "w": W,
            "cdiag": cdiag,
            "ident": ident,
            "coefb": np.repeat(c[None, :], PT, axis=0),
            "identb": np.eye(PT, dtype=ml_dtypes.bfloat16),
        })
    res = run_bass_kernel_spmd(nc, in_maps, list(range(NCORES)), trace=TRACE)
    LAST_RESULT = res
    return np.concatenate([res.results[i]["out"] for i in range(NCORES)], axis=0)



# revision 14
# speedup vs baseline: 1.4980x; 1.0145x over previous
"""GNN message-passing layer (GCN w/ edge-feature attention) on 8 trn2 cores.

Math (per graph b, N=512 nodes, E=8 edge feats, D=64):
    pre_sup = x_b @ W                                   [N, D]
    s[i,j]  = sum_e coef[e] * edge[b,i,j,e]             [N, N]
    adj     = softmax(s / tau, axis=-1)   (tau = 1.0)
    adj_hat = adj + I;  d = rowsum(adj_hat) = 2 exactly (softmax rows sum to 1)
    out     = relu(0.5 * adj_hat @ pre_sup)
            = relu( (P @ (0.5*pre_sup)) / Z + 0.5*pre_sup )
  where P = exp(s) (unnormalized, no max-subtraction needed: |s| <~ 25),
  Z_i = sum_j P[i,j] obtained for free as an extra ones-column in the
  aggregation matmul.

Device mapping (per core: 8 graphs, 64 MiB of edge data = the roofline):
  - scores (default v3b): 5 of the 8 e-terms as PSUM-accumulated PE matmuls
    (lhsT = coef[e]*I_128, rhs = stride-8 e-slice; fp32 matmul is 4 cyc/row,
    so splitting engines beats PE-only); the other 3 e-terms as ACT
    scaled-copies (per-partition scale AP = coef[e]) + DVE tree adds.
    NB: DVE tensor_tensor must not mix PSUM+SBUF operands (HW fault) -- the
    PE partial is tensor_copy'd out of PSUM before the final add.
  - exp: ACT engine (no max-subtraction needed; |scores| < ~25).
  - transpose P tiles on PE (is_transpose matmul vs identity), copy to SBUF
    on DVE, then aggregation matmuls contract j with rhs=[0.5*pre_sup | 1];
    the ones column yields the softmax denominator Z for free.
  - finals: reciprocal + per-partition scale + skip add + relu, DMA out.
  Cost-model (TimelineSim): ~238 us/iter, DMA-bound (DMA 200, PE 172,
  DVE 130, ACT 95 us busy); v1 (PE-only scores) was 269 us, PE-bound.
"""

import os
from contextlib import ExitStack

import ml_dtypes
import numpy as np

import concourse.bass as bass
import concourse.tile as tile
from concourse import bacc, mybir
from concourse.bass_utils import run_bass_kernel_spmd

F32 = mybir.dt.float32

B, N, E, D = 64, 512, 8, 64
NCORES = 8
BPC = B // NCORES          # graphs per core
PT = 128                   # partition tile (i-rows per edge tile)
NIT = N // PT              # 4 i-tiles (and j-chunks) per graph
TAU = 1.0

# Module-level knobs (test.py pokes these)
TRACE = os.environ.get("KERNEL_TRACE", "") == "1"
VARIANT = os.environ.get("KERNEL_VARIANT", "v3b")
LAST_RESULT = None

_nc_cache = {}


def _build_kernel(ctx: ExitStack, tc: "tile.TileContext", edge, x, w, cdiag, ident, out,
                  scores_f32r: bool = False, pe_e: int = E, coefb=None,
                  identb=None, edge_bufs: int = 4):
    """pe_e: how many of the E per-edge-feature score terms run as PE matmuls;
    the remaining E-pe_e run as ACT scaled-copies + DVE tree adds (fp32)."""
    nc = tc.nc
    EDT = mybir.dt.float32r if scores_f32r else F32

    consts = ctx.enter_context(tc.tile_pool(name="consts", bufs=1))
    if pe_e < E:
        acc_pool = ctx.enter_context(tc.tile_pool(name="acc", bufs=2))
        cb = consts.tile([PT, E], F32)
        nc.sync.dma_start(cb[:], coefb[:])
    edge_pool = ctx.enter_context(tc.tile_pool(name="edge", bufs=edge_bufs))
    xt_pool = ctx.enter_context(tc.tile_pool(name="xt", bufs=2))
    xT_pool = ctx.enter_context(tc.tile_pool(name="xT", bufs=2))
    psup_pool = ctx.enter_context(tc.tile_pool(name="psup", bufs=2))
    p_pool = ctx.enter_context(tc.tile_pool(name="p", bufs=2))
    pT_pool = ctx.enter_context(tc.tile_pool(name="pT", bufs=2))
    fin_pool = ctx.enter_context(tc.tile_pool(name="fin", bufs=3))
    o_pool = ctx.enter_context(tc.tile_pool(name="o", bufs=3))

    misc_psum = ctx.enter_context(tc.tile_pool(name="mpsum", bufs=2, space="PSUM"))
    sc_psum = ctx.enter_context(tc.tile_pool(name="scpsum", bufs=2, space="PSUM"))
    pT_psum = ctx.enter_context(tc.tile_pool(name="ptpsum", bufs=2, space="PSUM"))
    out_psum = ctx.enter_context(tc.tile_pool(name="opsum", bufs=2, space="PSUM"))

    # Constants
    cd = consts.tile([PT, E * PT], EDT)       # cd[:, e*128:(e+1)*128] = coef[e] * I
    if scores_f32r:
        nc.gpsimd.dma_start(cd[:], cdiag[:])  # SWDGE casts f32 -> f32r inline
    else:
        nc.sync.dma_start(cd[:], cdiag[:])
    idn = consts.tile([PT, PT], F32)
    nc.sync.dma_start(idn[:], ident[:])
    wsb = consts.tile([D, D], F32)
    nc.sync.dma_start(wsb[:], w[:])

    x_r = x[:].rearrange("(b it p) d -> b p it d", b=BPC, it=NIT, p=PT)

    def compute_psup(b):
        """pre_sup' = 0.5 * (x_b @ W) with a trailing ones column per j-chunk."""
        xt = xt_pool.tile([PT, NIT * D], F32)
        nc.sync.dma_start(xt[:].rearrange("p (it d) -> p it d", it=NIT), x_r[b])
        psup = psup_pool.tile([PT, NIT * (D + 1)], F32)
        for it in range(NIT):
            xT_ps = misc_psum.tile([D, PT], F32, tag="m")
            nc.tensor.matmul(xT_ps[:], xt[:, it * D:(it + 1) * D], idn[:],
                             is_transpose=True)
            xT_sb = xT_pool.tile([D, PT], F32)
            nc.vector.tensor_copy(xT_sb[:], xT_ps[:])
            ps_ps = misc_psum.tile([PT, D], F32, tag="m")
            nc.tensor.matmul(ps_ps[:], xT_sb[:], wsb[:], start=True, stop=True)
            nc.scalar.mul(psup[:, it * (D + 1):it * (D + 1) + D], ps_ps[:], 0.5)
            nc.vector.memset(psup[:, it * (D + 1) + D:(it + 1) * (D + 1)], 1.0)
        return psup

    def scores_tile(b, it):
        """DMA one edge tile and run the 8 accumulating score matmuls."""
        et = edge_pool.tile([PT, N * E], EDT)
        row0 = b * N + it * PT
        if scores_f32r:
            nc.gpsimd.dma_start(et[:], edge[row0:row0 + PT, :])
        else:
            nc.sync.dma_start(et[:], edge[row0:row0 + PT, :])
        et3 = et[:].rearrange("p (j e) -> p j e", e=E)
        sc_ps = sc_psum.tile([PT, N], F32)
        for e in range(pe_e):
            nc.tensor.matmul(sc_ps[:], cd[:, e * PT:(e + 1) * PT], et3[:, :, e],
                             start=(e == 0), stop=(e == pe_e - 1))
        p_sb = p_pool.tile([PT, N], F32)
        if pe_e == E:
            nc.scalar.activation(p_sb[:], sc_ps[:],
                                 mybir.ActivationFunctionType.Exp, scale=1.0 / TAU)
        else:
            # ACT: t_e = coef[e] * edge[:, :, e]; DVE: tree-add + fold in PSUM.
            ts = []
            for e in range(pe_e, E):
                t = acc_pool.tile([PT, N], F32, tag=f"t{e - pe_e}")
                nc.scalar.activation(t[:], et3[:, :, e],
                                     mybir.ActivationFunctionType.Copy,
                                     scale=cb[:, e:e + 1])
                ts.append(t)
            s = acc_pool.tile([PT, N], F32, tag="s0")
            nc.vector.tensor_add(s[:], ts[0][:], ts[1][:])
            for k, t in enumerate(ts[2:]):
                s2 = acc_pool.tile([PT, N], F32, tag=f"s{k + 1}")
                nc.vector.tensor_add(s2[:], s[:], t[:])
                s = s2
            # DVE tensor_tensor must not mix PSUM+SBUF operands (HW fault):
            # copy the PE partial out of PSUM first, then add SBUF+SBUF.
            sc_sb = acc_pool.tile([PT, N], F32, tag="scsb")
            nc.vector.tensor_copy(sc_sb[:], sc_ps[:])
            sf = acc_pool.tile([PT, N], F32, tag="sf")
            nc.vector.tensor_add(sf[:], sc_sb[:], s[:])
            nc.scalar.activation(p_sb[:], sf[:],
                                 mybir.ActivationFunctionType.Exp, scale=1.0 / TAU)
        return p_sb

    def post_tile(b, it, p_sb, psup):
        """Transpose P, aggregate against pre_sup'+ones, normalize, relu, store."""
        pT_sb = pT_pool.tile([PT, N], F32)
        for jc in range(NIT):
            pT_ps = pT_psum.tile([PT, PT], F32)
            nc.tensor.matmul(pT_ps[:], p_sb[:, jc * PT:(jc + 1) * PT], idn[:],
                             is_transpose=True)
            nc.vector.tensor_copy(pT_sb[:, jc * PT:(jc + 1) * PT], pT_ps[:])
        o_ps = out_psum.tile([PT, D + 1], F32)
        for jc in range(NIT):
            nc.tensor.matmul(o_ps[:], pT_sb[:, jc * PT:(jc + 1) * PT],
                             psup[:, jc * (D + 1):(jc + 1) * (D + 1)],
                             start=(jc == 0), stop=(jc == NIT - 1))
        r = fin_pool.tile([PT, 1], F32, tag="r")
        nc.vector.reciprocal(r[:], o_ps[:, D:D + 1])
        t1 = fin_pool.tile([PT, D], F32, tag="t1")
        nc.vector.tensor_scalar_mul(t1[:], o_ps[:, 0:D], r[:])
        t2 = fin_pool.tile([PT, D], F32, tag="t2")
        nc.vector.tensor_add(t2[:], t1[:],
                             psup[:, it * (D + 1):it * (D + 1) + D])
        o_sb = o_pool.tile([PT, D], F32)
        nc.scalar.activation(o_sb[:], t2[:], mybir.ActivationFunctionType.Relu)
        row0 = b * N + it * PT
        nc.sync.dma_start(out[row0:row0 + PT, :], o_sb[:])

    # Software-pipelined emission: post(k-1) lands between scores(k) and
    # scores(k+1) so the PE never waits on ACT's exp.
    pending = None
    for b in range(BPC):
        psup = compute_psup(b)
        for it in range(NIT):
            p_sb = scores_tile(b, it)
            if pending is not None:
                post_tile(*pending)
            pending = (b, it, p_sb, psup)
    post_tile(*pending)


def _build_kernel_v5(ctx: ExitStack, tc: "tile.TileContext", edge, x, w, cdiag, ident, out,
                     coefb=None, identb=None, p_bf16: bool = True,
                     edge_bufs: int = 4):
    """v5: all 8 score e-terms as PE matmuls with the f32 edge tile BITCAST to
    f32r (1 cyc/row at free-dim 512 vs 4 for fp32 — no cast DMA needed, f32r
    is bit-identical). P/psup in bf16 so transposes and the aggregation
    matmuls also run 1 cyc/row and the DVE copy traffic halves. Finals in f32
    (skip-add uses the f32 psup). PE busy drops ~4x vs v3b; the kernel should
    sit on the HBM stream."""
    nc = tc.nc
    F32R = mybir.dt.float32r
    BF16 = mybir.dt.bfloat16
    PDT = BF16 if p_bf16 else F32

    consts = ctx.enter_context(tc.tile_pool(name="consts", bufs=1))
    edge_pool = ctx.enter_context(tc.tile_pool(name="edge", bufs=edge_bufs))
    xt_pool = ctx.enter_context(tc.tile_pool(name="xt", bufs=2))
    xT_pool = ctx.enter_context(tc.tile_pool(name="xT", bufs=2))
    psup_pool = ctx.enter_context(tc.tile_pool(name="psup", bufs=2))
    psupb_pool = ctx.enter_context(tc.tile_pool(name="psupb", bufs=2))
    p_pool = ctx.enter_context(tc.tile_pool(name="p", bufs=2))
    pT_pool = ctx.enter_context(tc.tile_pool(name="pT", bufs=2))
    fin_pool = ctx.enter_context(tc.tile_pool(name="fin", bufs=3))
    o_pool = ctx.enter_context(tc.tile_pool(name="o", bufs=3))

    misc_psum = ctx.enter_context(tc.tile_pool(name="mpsum", bufs=2, space="PSUM"))
    sc_psum = ctx.enter_context(tc.tile_pool(name="scpsum", bufs=2, space="PSUM"))
    pT_psum = ctx.enter_context(tc.tile_pool(name="ptpsum", bufs=2, space="PSUM"))
    out_psum = ctx.enter_context(tc.tile_pool(name="opsum", bufs=2, space="PSUM"))

    # Constants: coef-scaled identities as f32r (bit-identical to f32), a f32
    # identity for the x transposes, a PDT identity for the P transposes.
    cd = consts.tile([PT, E * PT], F32R)
    nc.sync.dma_start(cd[:], cdiag[:].bitcast(F32R))
    idn = consts.tile([PT, PT], F32)
    nc.sync.dma_start(idn[:], ident[:])
    if p_bf16:
        idp = consts.tile([PT, PT], BF16)
        nc.sync.dma_start(idp[:], identb[:])
    else:
        idp = idn
    wsb = consts.tile([D, D], F32)
    nc.sync.dma_start(wsb[:], w[:])

    x_r = x[:].rearrange("(b it p) d -> b p it d", b=BPC, it=NIT, p=PT)

    def compute_psup(b):
        """pre_sup' = 0.5 * (x_b @ W); f32 (for the skip-add) + a PDT copy
        with a trailing ones column per j-chunk (for the aggregation)."""
        xt = xt_pool.tile([PT, NIT * D], F32)
        nc.sync.dma_start(xt[:].rearrange("p (it d) -> p it d", it=NIT), x_r[b])
        psup = psup_pool.tile([PT, NIT * D], F32)
        psupb = psupb_pool.tile([PT, NIT * (D + 1)], PDT)
        for it in range(NIT):
            xT_ps = misc_psum.tile([D, PT], F32, tag="m")
            nc.tensor.matmul(xT_ps[:], xt[:, it * D:(it + 1) * D], idn[:],
                             is_transpose=True)
            xT_sb = xT_pool.tile([D, PT], F32)
            nc.vector.tensor_copy(xT_sb[:], xT_ps[:])
            ps_ps = misc_psum.tile([PT, D], F32, tag="m")
            nc.tensor.matmul(ps_ps[:], xT_sb[:], wsb[:], start=True, stop=True)
            nc.scalar.mul(psup[:, it * D:(it + 1) * D], ps_ps[:], 0.5)
            nc.vector.tensor_copy(psupb[:, it * (D + 1):it * (D + 1) + D],
                                  psup[:, it * D:(it + 1) * D])
            nc.vector.memset(psupb[:, it * (D + 1) + D:(it + 1) * (D + 1)], 1.0)
        return psup, psupb

    def scores_tile(b, it):
        """DMA one edge tile; 8 accumulating f32r score matmuls; exp -> PDT."""
        et = edge_pool.tile([PT, N * E], F32R)
        row0 = b * N + it * PT
        nc.sync.dma_start(et[:], edge[row0:row0 + PT, :].bitcast(F32R))
        et3 = et[:].rearrange("p (j e) -> p j e", e=E)
        sc_ps = sc_psum.tile([PT, N], F32)
        for e in range(E):
            nc.tensor.matmul(sc_ps[:], cd[:, e * PT:(e + 1) * PT], et3[:, :, e],
                             start=(e == 0), stop=(e == E - 1))
        p_sb = p_pool.tile([PT, N], PDT)
        nc.scalar.activation(p_sb[:], sc_ps[:],
                             mybir.ActivationFunctionType.Exp, scale=1.0 / TAU)
        return p_sb

    def post_tile(b, it, p_sb, psup, psupb):
        """Transpose P, aggregate against pre_sup'+ones, normalize, relu, store."""
        pT_sb = pT_pool.tile([PT, N], PDT)
        for jc in range(NIT):
            pT_ps = pT_psum.tile([PT, PT], PDT)
            nc.tensor.matmul(pT_ps[:], p_sb[:, jc * PT:(jc + 1) * PT], idp[:],
                             is_transpose=True)
            nc.vector.tensor_copy(pT_sb[:, jc * PT:(jc + 1) * PT], pT_ps[:])
        o_ps = out_psum.tile([PT, D + 1], F32)
        for jc in range(NIT):
            nc.tensor.matmul(o_ps[:], pT_sb[:, jc * PT:(jc + 1) * PT],
                             psupb[:, jc * (D + 1):(jc + 1) * (D + 1)],
                             start=(jc == 0), stop=(jc == NIT - 1))
        r = fin_pool.tile([PT, 1], F32, tag="r")
        nc.vector.reciprocal(r[:], o_ps[:, D:D + 1])
        t1 = fin_pool.tile([PT, D], F32, tag="t1")
        nc.vector.tensor_scalar_mul(t1[:], o_ps[:, 0:D], r[:])
        t2 = fin_pool.tile([PT, D], F32, tag="t2")
        nc.vector.tensor_add(t2[:], t1[:], psup[:, it * D:(it + 1) * D])
        o_sb = o_pool.tile([PT, D], F32)
        nc.scalar.activation(o_sb[:], t2[:], mybir.ActivationFunctionType.Relu)
        row0 = b * N + it * PT
        nc.sync.dma_start(out[row0:row0 + PT, :], o_sb[:])

    pending = None
    for b in range(BPC):
        psup, psupb = compute_psup(b)
        for it in range(NIT):
            p_sb = scores_tile(b, it)
            if pending is not None:
                post_tile(*pending)
            pending = (b, it, p_sb, psup, psupb)
    post_tile(*pending)


def _build_kernel_v6(ctx: ExitStack, tc: "tile.TileContext", edge, x, w, cdiag, ident, out,
                     coefb=None, identb=None, hsplit: int = 2, edge_bufs: int = 8):
    """v6 = v5 with each 2 MiB edge tile split into `hsplit` j-range chunks.
    Finer DMA/compute interleave: PE gets work every ~DMA-chunk instead of
    bursting once per 2 MiB tile, which keeps the HAM clock-gate warm (v5
    measured 210 us throttled at K=4/8) and shortens the pipeline tail."""
    nc = tc.nc
    F32R = mybir.dt.float32r
    BF16 = mybir.dt.bfloat16
    PDT = BF16
    JW = N // hsplit              # j-width per chunk (>=256 keeps f32r 1 cyc/row)
    JB = JW // PT                 # 128-blocks per chunk

    consts = ctx.enter_context(tc.tile_pool(name="consts", bufs=1))
    edge_pool = ctx.enter_context(tc.tile_pool(name="edge", bufs=edge_bufs))
    xt_pool = ctx.enter_context(tc.tile_pool(name="xt", bufs=2))
    xT_pool = ctx.enter_context(tc.tile_pool(name="xT", bufs=2))
    psup_pool = ctx.enter_context(tc.tile_pool(name="psup", bufs=2))
    psupb_pool = ctx.enter_context(tc.tile_pool(name="psupb", bufs=2))
    p_pool = ctx.enter_context(tc.tile_pool(name="p", bufs=3))
    pT_pool = ctx.enter_context(tc.tile_pool(name="pT", bufs=3))
    fin_pool = ctx.enter_context(tc.tile_pool(name="fin", bufs=3))
    o_pool = ctx.enter_context(tc.tile_pool(name="o", bufs=3))

    misc_psum = ctx.enter_context(tc.tile_pool(name="mpsum", bufs=2, space="PSUM"))
    sc_psum = ctx.enter_context(tc.tile_pool(name="scpsum", bufs=2, space="PSUM"))
    pT_psum = ctx.enter_context(tc.tile_pool(name="ptpsum", bufs=2, space="PSUM"))
    out_psum = ctx.enter_context(tc.tile_pool(name="opsum", bufs=2, space="PSUM"))

    cd = consts.tile([PT, E * PT], F32R)
    nc.sync.dma_start(cd[:], cdiag[:].bitcast(F32R))
    idn = consts.tile([PT, PT], F32)
    nc.sync.dma_start(idn[:], ident[:])
    idp = consts.tile([PT, PT], BF16)
    nc.sync.dma_start(idp[:], identb[:])
    wsb = consts.tile([D, D], F32)
    nc.sync.dma_start(wsb[:], w[:])

    x_r = x[:].rearrange("(b it p) d -> b p it d", b=BPC, it=NIT, p=PT)

    def compute_psup(b):
        xt = xt_pool.tile([PT, NIT * D], F32)
        nc.sync.dma_start(xt[:].rearrange("p (it d) -> p it d", it=NIT), x_r[b])
        psup = psup_pool.tile([PT, NIT * D], F32)
        psupb = psupb_pool.tile([PT, NIT * (D + 1)], PDT)
        for it in range(NIT):
            xT_ps = misc_psum.tile([D, PT], F32, tag="m")
            nc.tensor.matmul(xT_ps[:], xt[:, it * D:(it + 1) * D], idn[:],
                             is_transpose=True)
            xT_sb = xT_pool.tile([D, PT], F32)
            nc.vector.tensor_copy(xT_sb[:], xT_ps[:])
            ps_ps = misc_psum.tile([PT, D], F32, tag="m")
            nc.tensor.matmul(ps_ps[:], xT_sb[:], wsb[:], start=True, stop=True)
            nc.scalar.mul(psup[:, it * D:(it + 1) * D], ps_ps[:], 0.5)
            nc.vector.tensor_copy(psupb[:, it * (D + 1):it * (D + 1) + D],
                                  psup[:, it * D:(it + 1) * D])
            nc.vector.memset(psupb[:, it * (D + 1) + D:(it + 1) * (D + 1)], 1.0)
        return psup, psupb

    def scores_chunk(b, it, h):
        """DMA one [PT, JW*E] edge chunk; 8 accumulating f32r matmuls; exp."""
        et = edge_pool.tile([PT, JW * E], F32R)
        row0 = b * N + it * PT
        nc.sync.dma_start(et[:],
                          edge[row0:row0 + PT, h * JW * E:(h + 1) * JW * E]
                          .bitcast(F32R))
        et3 = et[:].rearrange("p (j e) -> p j e", e=E)
        sc_ps = sc_psum.tile([PT, JW], F32)
        for e in range(E):
            nc.tensor.matmul(sc_ps[:], cd[:, e * PT:(e + 1) * PT], et3[:, :, e],
                             start=(e == 0), stop=(e == E - 1))
        p_sb = p_pool.tile([PT, JW], PDT)
        nc.scalar.activation(p_sb[:], sc_ps[:],
                             mybir.ActivationFunctionType.Exp, scale=1.0 / TAU)
        return p_sb

    def post_chunk(b, it, h, p_sb, psupb, o_ps):
        """Transpose this chunk's P blocks and fold them into o_ps."""
        pT_sb = pT_pool.tile([PT, JW], PDT)
        for jj in range(JB):
            pT_ps = pT_psum.tile([PT, PT], PDT)
            nc.tensor.matmul(pT_ps[:], p_sb[:, jj * PT:(jj + 1) * PT], idp[:],
                             is_transpose=True)
            nc.vector.tensor_copy(pT_sb[:, jj * PT:(jj + 1) * PT], pT_ps[:])
        for jj in range(JB):
            jc = h * JB + jj
            nc.tensor.matmul(o_ps[:], pT_sb[:, jj * PT:(jj + 1) * PT],
                             psupb[:, jc * (D + 1):(jc + 1) * (D + 1)],
                             start=(jc == 0), stop=(jc == NIT - 1))

    def finals(b, it, psup, o_ps):
        r = fin_pool.tile([PT, 1], F32, tag="r")
        nc.vector.reciprocal(r[:], o_ps[:, D:D + 1])
        t1 = fin_pool.tile([PT, D], F32, tag="t1")
        nc.vector.tensor_scalar_mul(t1[:], o_ps[:, 0:D], r[:])
        t2 = fin_pool.tile([PT, D], F32, tag="t2")
        nc.vector.tensor_add(t2[:], t1[:], psup[:, it * D:(it + 1) * D])
        o_sb = o_pool.tile([PT, D], F32)
        nc.scalar.activation(o_sb[:], t2[:], mybir.ActivationFunctionType.Relu)
        row0 = b * N + it * PT
        nc.sync.dma_start(out[row0:row0 + PT, :], o_sb[:])

    pending = None       # (b, it, h, p_sb, psupb, o_ps)
    fin_pending = None   # (b, it, psup, o_ps)
    for b in range(BPC):
        psup, psupb = compute_psup(b)
        for it in range(NIT):
            o_ps = out_psum.tile([PT, D + 1], F32)
            for h in range(hsplit):
                p_sb = scores_chunk(b, it, h)
                if pending is not None:
                    post_chunk(*pending)
                    if pending[2] == hsplit - 1:
                        finals(*fin_pending)
                pending = (b, it, h, p_sb, psupb, o_ps)
                if h == hsplit - 1:
                    fin_pending = (b, it, psup, o_ps)
    post_chunk(*pending)
    finals(*fin_pending)


def _build_dma_only(ctx: ExitStack, tc: "tile.TileContext", edge, x, w, cdiag, ident, out,
                    coefb=None, identb=None):
    """Variant: just the edge DMA stream + a trivial out write (BW probe)."""
    nc = tc.nc
    edge_pool = ctx.enter_context(tc.tile_pool(name="edge", bufs=4))
    o_pool = ctx.enter_context(tc.tile_pool(name="o", bufs=2))
    for b in range(BPC):
        for it in range(NIT):
            et = edge_pool.tile([PT, N * E], F32)
            row0 = b * N + it * PT
            nc.sync.dma_start(et[:], edge[row0:row0 + PT, :])
            o_sb = o_pool.tile([PT, D], F32)
            nc.vector.tensor_copy(o_sb[:], et[:, 0:D])
            nc.sync.dma_start(out[row0:row0 + PT, :], o_sb[:])


def _build_kernel_f32r(ctx, tc, edge, x, w, cdiag, ident, out, coefb=None, identb=None):
    _build_kernel(ctx, tc, edge, x, w, cdiag, ident, out, scores_f32r=True)


def _build_kernel_split4(ctx, tc, edge, x, w, cdiag, ident, out, coefb=None, identb=None):
    _build_kernel(ctx, tc, edge, x, w, cdiag, ident, out, pe_e=4, coefb=coefb)


def _build_kernel_split5(ctx, tc, edge, x, w, cdiag, ident, out, coefb=None, identb=None):
    _build_kernel(ctx, tc, edge, x, w, cdiag, ident, out, pe_e=5, coefb=coefb)


def _build_kernel_v4(ctx, tc, edge, x, w, cdiag, ident, out, coefb=None, identb=None):
    _build_kernel(ctx, tc, edge, x, w, cdiag, ident, out, pe_e=5, coefb=coefb,
                  edge_bufs=6)


def _build_kernel_v5f(ctx, tc, edge, x, w, cdiag, ident, out, coefb=None, identb=None):
    _build_kernel_v5(ctx, tc, edge, x, w, cdiag, ident, out, identb=identb,
                     p_bf16=False)


def _build_kernel_v6b(ctx, tc, edge, x, w, cdiag, ident, out, coefb=None, identb=None):
    _build_kernel_v6(ctx, tc, edge, x, w, cdiag, ident, out, identb=identb,
                     hsplit=2, edge_bufs=12)


def _build_kernel_v7(ctx: ExitStack, tc: "tile.TileContext", edge, x, w, cdiag, ident, out,
                     coefb=None, identb=None, edge_bufs: int = 8):
    """v7 = v5 + DMA-queue separation. The v6 trace showed the edge stream
    (8 KiB packets) sharing its HWDGE queue with ~8k 256-byte packets from the
    x loads and per-tile out writes; the round-robin stole ~25% of SDMA engine
    time. Here the sync-engine ring carries ONLY the 64 MiB edge stream; all
    small traffic (consts, one 1 MiB x preload, per-graph batched out writes)
    issues on the scalar-engine HWDGE ring."""
    nc = tc.nc
    F32R = mybir.dt.float32r
    BF16 = mybir.dt.bfloat16
    PDT = BF16

    consts = ctx.enter_context(tc.tile_pool(name="consts", bufs=1))
    edge_pool = ctx.enter_context(tc.tile_pool(name="edge", bufs=edge_bufs))
    xT_pool = ctx.enter_context(tc.tile_pool(name="xT", bufs=2))
    psup_pool = ctx.enter_context(tc.tile_pool(name="psup", bufs=2))
    psupb_pool = ctx.enter_context(tc.tile_pool(name="psupb", bufs=2))
    p_pool = ctx.enter_context(tc.tile_pool(name="p", bufs=2))
    pT_pool = ctx.enter_context(tc.tile_pool(name="pT", bufs=2))
    fin_pool = ctx.enter_context(tc.tile_pool(name="fin", bufs=3))
    o_pool = ctx.enter_context(tc.tile_pool(name="o", bufs=2))

    misc_psum = ctx.enter_context(tc.tile_pool(name="mpsum", bufs=2, space="PSUM"))
    sc_psum = ctx.enter_context(tc.tile_pool(name="scpsum", bufs=2, space="PSUM"))
    pT_psum = ctx.enter_context(tc.tile_pool(name="ptpsum", bufs=2, space="PSUM"))
    out_psum = ctx.enter_context(tc.tile_pool(name="opsum", bufs=2, space="PSUM"))

    cd = consts.tile([PT, E * PT], F32R)
    nc.scalar.dma_start(cd[:], cdiag[:].bitcast(F32R))
    idn = consts.tile([PT, PT], F32)
    nc.scalar.dma_start(idn[:], ident[:])
    idp = consts.tile([PT, PT], BF16)
    nc.scalar.dma_start(idp[:], identb[:])
    wsb = consts.tile([D, D], F32)
    nc.scalar.dma_start(wsb[:], w[:])
    # One-shot x preload: [p, (b it d)] = 8 KiB/partition.
    xall = consts.tile([PT, BPC * NIT * D], F32)
    nc.scalar.dma_start(
        xall[:].rearrange("p (b it d) -> p b it d", b=BPC, it=NIT),
        x[:].rearrange("(b it p) d -> p b it d", b=BPC, it=NIT, p=PT))

    def compute_psup(b):
        psup = psup_pool.tile([PT, NIT * D], F32)
        psupb = psupb_pool.tile([PT, NIT * (D + 1)], PDT)
        for it in range(NIT):
            col0 = (b * NIT + it) * D
            xT_ps = misc_psum.tile([D, PT], F32, tag="m")
            nc.tensor.matmul(xT_ps[:], xall[:, col0:col0 + D], idn[:],
                             is_transpose=True)
            xT_sb = xT_pool.tile([D, PT], F32)
            nc.vector.tensor_copy(xT_sb[:], xT_ps[:])
            ps_ps = misc_psum.tile([PT, D], F32, tag="m")
            nc.tensor.matmul(ps_ps[:], xT_sb[:], wsb[:], start=True, stop=True)
            nc.scalar.mul(psup[:, it * D:(it + 1) * D], ps_ps[:], 0.5)
            nc.vector.tensor_copy(psupb[:, it * (D + 1):it * (D + 1) + D],
                                  psup[:, it * D:(it + 1) * D])
            nc.vector.memset(psupb[:, it * (D + 1) + D:(it + 1) * (D + 1)], 1.0)
        return psup, psupb

    def scores_tile(b, it):
        et = edge_pool.tile([PT, N * E], F32R)
        row0 = b * N + it * PT
        nc.sync.dma_start(et[:], edge[row0:row0 + PT, :].bitcast(F32R))
        et3 = et[:].rearrange("p (j e) -> p j e", e=E)
        sc_ps = sc_psum.tile([PT, N], F32)
        for e in range(E):
            nc.tensor.matmul(sc_ps[:], cd[:, e * PT:(e + 1) * PT], et3[:, :, e],
                             start=(e == 0), stop=(e == E - 1))
        p_sb = p_pool.tile([PT, N], PDT)
        nc.scalar.activation(p_sb[:], sc_ps[:],
                             mybir.ActivationFunctionType.Exp, scale=1.0 / TAU)
        return p_sb

    def post_tile(b, it, p_sb, psup, psupb, ob):
        pT_sb = pT_pool.tile([PT, N], PDT)
        for jc in range(NIT):
            pT_ps = pT_psum.tile([PT, PT], PDT)
            nc.tensor.matmul(pT_ps[:], p_sb[:, jc * PT:(jc + 1) * PT], idp[:],
                             is_transpose=True)
            nc.vector.tensor_copy(pT_sb[:, jc * PT:(jc + 1) * PT], pT_ps[:])
        o_ps = out_psum.tile([PT, D + 1], F32)
        for jc in range(NIT):
            nc.tensor.matmul(o_ps[:], pT_sb[:, jc * PT:(jc + 1) * PT],
                             psupb[:, jc * (D + 1):(jc + 1) * (D + 1)],
                             start=(jc == 0), stop=(jc == NIT - 1))
        r = fin_pool.tile([PT, 1], F32, tag="r")
        nc.vector.reciprocal(r[:], o_ps[:, D:D + 1])
        t1 = fin_pool.tile([PT, D], F32, tag="t1")
        nc.vector.tensor_scalar_mul(t1[:], o_ps[:, 0:D], r[:])
        t2 = fin_pool.tile([PT, D], F32, tag="t2")
        nc.vector.tensor_add(t2[:], t1[:], psup[:, it * D:(it + 1) * D])
        nc.scalar.activation(ob[:, it * D:(it + 1) * D], t2[:],
                             mybir.ActivationFunctionType.Relu)
        if it == NIT - 1:
            nc.scalar.dma_start(
                out[b * N:(b + 1) * N, :].rearrange("(it p) d -> p it d",
                                                    it=NIT, p=PT),
                ob[:].rearrange("p (it d) -> p it d", it=NIT))

    pending = None
    for b in range(BPC):
        psup, psupb = compute_psup(b)
        ob = o_pool.tile([PT, NIT * D], F32)
        for it in range(NIT):
            p_sb = scores_tile(b, it)
            if pending is not None:
                post_tile(*pending)
            pending = (b, it, p_sb, psup, psupb, ob)
    post_tile(*pending)


_BUILDERS = {"v1": _build_kernel, "v2": _build_kernel_f32r, "dma": _build_dma_only,
             "v3": _build_kernel_split4, "v3b": _build_kernel_split5,
             "v4": _build_kernel_v4, "v5": _build_kernel_v5,
             "v5f": _build_kernel_v5f, "v6": _build_kernel_v6,
             "v6b": _build_kernel_v6b, "v7": _build_kernel_v7}


def _get_nc(reps: int = 1, variant: str = "v1"):
    key = f"{variant}-r{reps}"
    internal_edge = variant.endswith("i")
    base_variant = variant[:-1] if internal_edge else variant
    if key not in _nc_cache:
        nc = bacc.Bacc("TRN2", target_bir_lowering=False, debug=False,
                       num_devices=NCORES)
        if internal_edge:
            # Bench-only: edge lives in device DRAM (uninitialized) so the
            # axon tunnel doesn't re-ship 512 MiB per timed call.
            edge = nc.dram_tensor("edge_int", [BPC * N, N * E], F32)
        else:
            edge = nc.declare_dram_parameter("edge", [BPC * N, N * E], F32, isOutput=False)
        x = nc.declare_dram_parameter("x", [BPC * N, D], F32, isOutput=False)
        w = nc.declare_dram_parameter("w", [D, D], F32, isOutput=False)
        cdiag = nc.declare_dram_parameter("cdiag", [PT, E * PT], F32, isOutput=False)
        ident = nc.declare_dram_parameter("ident", [PT, PT], F32, isOutput=False)
        coefb = nc.declare_dram_parameter("coefb", [PT, E], F32, isOutput=False)
        identb = nc.declare_dram_parameter("identb", [PT, PT], mybir.dt.bfloat16,
                                           isOutput=False)
        out = nc.declare_dram_parameter("out", [BPC * N, D], F32, isOutput=True)
        builder = _BUILDERS[base_variant]
        with tile.TileContext(nc) as tc:
            for _ in range(reps):
                with ExitStack() as ctx:
                    builder(ctx, tc, edge, x, w, cdiag, ident, out, coefb=coefb,
                            identb=identb)
        nc.compile()
        _nc_cache[key] = nc
    return _nc_cache[key]


def kernel(**inputs) -> np.ndarray:
    global LAST_RESULT
    edge = np.ascontiguousarray(inputs["edge_features"], dtype=np.float32)
    x = np.ascontiguousarray(inputs["x"], dtype=np.float32)
    W = np.ascontiguousarray(inputs["W"], dtype=np.float32)
    coef = np.asarray(inputs["coef"], dtype=np.float32)

    c = coef[:, 0]
    cdiag = np.zeros((PT, E * PT), np.float32)
    ar = np.arange(PT)
    for e in range(E):
        cdiag[ar, e * PT + ar] = c[e]
    ident = np.eye(PT, dtype=np.float32)

    nc = _get_nc(variant=VARIANT)
    in_maps = []
    for core in range(NCORES):
        b0 = core * BPC
        in_maps.append({
            "edge": edge[b0:b0 + BPC].reshape(BPC * N, N * E),
            "x": x[b0 * N:(b0 + BPC) * N],
            "w": W,
            "cdiag": cdiag,
            "ident": ident,
            "coefb": np.repeat(c[None, :], PT, axis=0),
            "identb": np.eye(PT, dtype=ml_dtypes.bfloat16),
        })
    res = run_bass_kernel_spmd(nc, in_maps, list(range(NCORES)), trace=TRACE)
    LAST_RESULT = res
    return np.concatenate([res.results[i]["out"] for i in range(NCORES)], axis=0)

